# revision 9
# baseline (speedup 1.0000x reference)
"""Trainium2 Bass kernel for a dense transformer block:
x -> LN1 -> causal MHA (16 heads) -> +residual -> LN2 -> FFN(4x, relu) -> +residual

Full inputs in, full outputs out. Sharding: 8 cores = (batch b in 0..3) x (parity p in 0..1).
Core (b, p) owns query blocks {2j+p : j in 0..3} of 256 tokens of batch b (1024 tokens),
computes K/V for the whole batch (duplicated within the pair), runs block-causal attention
with a uniform SPMD program (per-core causal masks passed as data), then proj/LN2/FFN on its
own token rows. No collectives. Matmuls in float32r (TF32-like, 1 cyc/row at free-dim>=256).

Layout notes:
 - "T" suffix = transposed layout [feature, token]; LN is computed in transposed layout with
   per-token stats built via ones-matmuls, broadcast across partitions on GpSimd.
 - LN applies in-place (xT tile becomes hT) to save SBUF; big slabs share pool tags so later
   phases (FFN uT) reuse attention-phase SBUF.
 - x2 (post-attention residual) is spilled to DRAM and re-read by LN2/FFN.
"""

import numpy as np

B, T, D = 4, 2048, 1024
H, DH = 16, 64
NG = 8            # head groups of 2 heads
TC = 1024         # tokens per core
QB = 256          # query block
NJ = 4            # local query blocks per core
F4 = 4096
EPS = 1e-5
SCALE = float(D) ** -0.5
N_CORES = 8

_cache = {}


def _build():
    import contextlib
    import concourse.bass as bass
    import concourse.mybir as mybir
    import concourse.tile as tile
    from concourse import bacc
    from concourse.masks import make_identity

    f32, f32r = mybir.dt.float32, mybir.dt.float32r
    AF = mybir.ActivationFunctionType
    ALU = mybir.AluOpType

    nc = bacc.Bacc('TRN2', target_bir_lowering=False, debug=False,
                   num_devices=N_CORES)

    # ---- external I/O (per-core) ----
    xT_d = nc.dram_tensor("xT", [D, T], f32, kind="ExternalInput")
    xqT_d = nc.dram_tensor("xqT", [D, TC], f32, kind="ExternalInput")
    xq_d = nc.dram_tensor("xq", [TC, D], f32, kind="ExternalInput")
    wq_d = nc.dram_tensor("wqp", [NG, 8, 128, 128], f32, kind="ExternalInput")
    wk_d = nc.dram_tensor("wkp", [NG, 8, 128, 128], f32, kind="ExternalInput")
    wv_d = nc.dram_tensor("wvp", [NG, 8, 128, 128], f32, kind="ExternalInput")
    wp_d = nc.dram_tensor("wp", [D, D], f32, kind="ExternalInput")
    w1_d = nc.dram_tensor("w1p", [32, 8, 128, 128], f32, kind="ExternalInput")
    w2_d = nc.dram_tensor("w2", [F4, D], f32, kind="ExternalInput")
    g1_d = nc.dram_tensor("g1t", [8, 128], f32, kind="ExternalInput")
    be1_d = nc.dram_tensor("be1t", [8, 128], f32, kind="ExternalInput")
    g2_d = nc.dram_tensor("g2t", [8, 128], f32, kind="ExternalInput")
    be2_d = nc.dram_tensor("be2t", [8, 128], f32, kind="ExternalInput")
    b1_d = nc.dram_tensor("b1t", [32, 128], f32, kind="ExternalInput")
    bp_d = nc.dram_tensor("bp", [D], f32, kind="ExternalInput")
    b2_d = nc.dram_tensor("b2", [D], f32, kind="ExternalInput")
    mk_d = nc.dram_tensor("masks", [4, 128, QB], f32, kind="ExternalInput")
    out_d = nc.dram_tensor("out", [TC, D], f32, kind="ExternalOutput")

    x2_dram = nc.dram_tensor("x2_scratch", [TC, D], f32)
    den_dram = nc.dram_tensor("den_scratch", [2, TC], f32)
    rden_dram = nc.dram_tensor("rden_scratch", [2, TC], f32)

    def bcast_ap(dram_ap, parts, n):
        return bass.AP(tensor=dram_ap.tensor, offset=dram_ap.offset,
                       ap=[[0, parts], [1, n]])

    with tile.TileContext(nc) as tc:
        ctx = contextlib.ExitStack()
        with ctx:
            consts = ctx.enter_context(tc.tile_pool(name="consts", bufs=1))
            persist = ctx.enter_context(tc.tile_pool(name="persist", bufs=1))
            # ---------- constants ----------
            g1t = consts.tile([128, 8], f32)
            nc.sync.dma_start(out=g1t, in_=g1_d[:, :].rearrange("g p -> p g"))
            be1t = consts.tile([128, 8], f32)
            nc.sync.dma_start(out=be1t, in_=be1_d[:, :].rearrange("g p -> p g"))
            g2t = consts.tile([128, 8], f32)
            nc.sync.dma_start(out=g2t, in_=g2_d[:, :].rearrange("g p -> p g"))
            be2t = consts.tile([128, 8], f32)
            nc.sync.dma_start(out=be2t, in_=be2_d[:, :].rearrange("g p -> p g"))
            ident_f = consts.tile([128, 128], f32)
            make_identity(nc, ident_f)
            ident_r = consts.tile([128, 128], f32r)
            nc.vector.tensor_copy(ident_r, ident_f)
            ones_f = consts.tile([128, 16], f32)
            nc.vector.memset(ones_f, 1.0)
            ones_r = consts.tile([128, 1], f32r)
            nc.vector.tensor_copy(ones_r, ones_f[:, 0:1])
            eps_t = consts.tile([1, 1], f32)
            nc.vector.memset(eps_t, EPS)

            # ---------- transposed layernorm (in-place capable) ----------
            def ln_T(src_all, dst_all, n_tok, gt, bt, wpool, spool, pspool):
                nch = n_tok // 512
                for c in range(nch):
                    sl = bass.ds(c * 512, 512)
                    mu_ps = pspool.tile([1, 512], f32, tag="mu_ps")
                    sq_ps = pspool.tile([1, 512], f32, tag="sq_ps")
                    for i in range(8):
                        sq = wpool.tile([128, 512], f32r, tag="sq")
                        nc.scalar.activation(out=sq, in_=src_all[:, i, sl].bitcast(f32),
                                             func=AF.Square)
                        nc.tensor.matmul(mu_ps, ones_r, src_all[:, i, sl],
                                         start=(i == 0), stop=(i == 7))
                        nc.tensor.matmul(sq_ps, ones_r, sq,
                                         start=(i == 0), stop=(i == 7))
                    mu = spool.tile([1, 512], f32, tag="mu")
                    nc.scalar.mul(mu, mu_ps, 1.0 / D)
                    sb = spool.tile([1, 512], f32, tag="sb")
                    nc.scalar.mul(sb, sq_ps, 1.0 / D)
                    sc = spool.tile([1, 512], f32, tag="sc")
                    nc.vector.tensor_tensor(out=sc, in0=mu, in1=mu, op=ALU.mult)
                    nc.vector.tensor_tensor(out=sb, in0=sb, in1=sc, op=ALU.subtract)
                    nc.scalar.activation(out=sb, in_=sb, func=AF.Sqrt, bias=eps_t)
                    nc.vector.reciprocal(sc, sb)
                    sd_ = spool.tile([1, 512], f32, tag="sd")
                    nc.vector.tensor_tensor(out=sd_, in0=sb, in1=sc, op=ALU.mult)
                    nc.vector.tensor_scalar(out=sd_, in0=sd_, scalar1=-1.0,
                                            scalar2=2.0, op0=ALU.mult, op1=ALU.add)
                    nc.vector.tensor_tensor(out=sd_, in0=sc, in1=sd_, op=ALU.mult)
                    mu_b = wpool.tile([128, 512], f32, tag="mu_b")
                    nc.gpsimd.partition_broadcast(mu_b, mu)
                    rstd_b = wpool.tile([128, 512], f32, tag="rstd_b")
                    nc.gpsimd.partition_broadcast(rstd_b, sd_)
                    for i in range(8):
                        t1 = wpool.tile([128, 512], f32, tag="t1")
                        nc.vector.tensor_tensor(out=t1,
                                                in0=src_all[:, i, sl].bitcast(f32),
                                                in1=mu_b, op=ALU.subtract)
                        nc.vector.tensor_tensor(out=t1, in0=t1, in1=rstd_b,
                                                op=ALU.mult)
                        nc.vector.tensor_scalar(out=dst_all[:, i, sl], in0=t1,
                                                scalar1=gt[:, i:i + 1],
                                                scalar2=bt[:, i:i + 1],
                                                op0=ALU.mult, op1=ALU.add)

            # ---------- LN1: xT -> hT (in place), xqT -> hqT (in place) ----------
            # persist tags: t64: hT -> uT(x2) ; t32a: hqT -> x2T ; t32b: attT
            hT = persist.tile([128, 8, T], f32r, tag="t64")
            hqT = persist.tile([128, 8, TC], f32r, tag="t32a")
            with tc.tile_pool(name="ln_work", bufs=3) as lnw, \
                 tc.tile_pool(name="ln_stats", bufs=1) as lns, \
                 tc.tile_pool(name="ps_ln1", bufs=1, space="PSUM") as ps_ln1:
                for i in range(8):
                    nc.sync.dma_start(
                        out=hT[:, i, :],
                        in_=xT_d[i * 128:(i + 1) * 128, :].bitcast(f32r))
                ln_T(hT, hT, T, g1t, be1t, lnw, lns, ps_ln1)
                for i in range(8):
                    nc.sync.dma_start(
                        out=hqT[:, i, :],
                        in_=xqT_d[i * 128:(i + 1) * 128, :].bitcast(f32r))
                ln_T(hqT, hqT, TC, g1t, be1t, lnw, lns, ps_ln1)

            # ---------- attention ----------
            attT = persist.tile([128, 8, TC], f32r, tag="t32b")
            with tc.tile_pool(name="kv_pool", bufs=1) as kvp, \
                 tc.tile_pool(name="wg_pool", bufs=1) as wgp, \
                 tc.tile_pool(name="att_work", bufs=4) as atw, \
                 tc.tile_pool(name="den_pool", bufs=1) as dnp, \
                 tc.tile_pool(name="ps_qkv", bufs=2, space="PSUM") as ps_qkv, \
                 tc.tile_pool(name="ps_st", bufs=3, space="PSUM") as ps_st, \
                 tc.tile_pool(name="ps_tr", bufs=1, space="PSUM") as ps_tr, \
                 tc.tile_pool(name="ps_acc", bufs=1, space="PSUM") as ps_acc:
                mask_t = consts.tile([128, 4, QB], f32)
                nc.sync.dma_start(out=mask_t,
                                  in_=mk_d[:, :, :].rearrange("r p q -> p r q"))
                for g in range(NG):
                    wqg = wgp.tile([128, 8, 128], f32r, tag="wqg")
                    nc.sync.dma_start(
                        out=wqg, in_=wq_d[g].rearrange("k p c -> p k c").bitcast(f32r))
                    wkg = wgp.tile([128, 8, 128], f32r, tag="wkg")
                    nc.sync.dma_start(
                        out=wkg, in_=wk_d[g].rearrange("k p c -> p k c").bitcast(f32r))
                    wvg = wgp.tile([128, 8, 128], f32r, tag="wvg")
                    nc.sync.dma_start(
                        out=wvg, in_=wv_d[g].rearrange("k p c -> p k c").bitcast(f32r))
                    kt = kvp.tile([128, T], f32r, tag="kt")
                    vt = kvp.tile([128, T], f32r, tag="vt")
                    qt = kvp.tile([128, TC], f32r, tag="qt")
                    for n in range(4):
                        sl = bass.ds(n * 512, 512)
                        psk = ps_qkv.tile([128, 512], f32, tag="qkv")
                        for k in range(8):
                            nc.tensor.matmul(psk, wkg[:, k, :], hT[:, k, sl],
                                             start=(k == 0), stop=(k == 7))
                        nc.scalar.copy(kt[:, sl], psk)
                        psv = ps_qkv.tile([128, 512], f32, tag="qkv")
                        for k in range(8):
                            nc.tensor.matmul(psv, wvg[:, k, :], hT[:, k, sl],
                                             start=(k == 0), stop=(k == 7))
                        nc.vector.tensor_copy(vt[:, sl], psv)
                    for n in range(2):
                        sl = bass.ds(n * 512, 512)
                        psq = ps_qkv.tile([128, 512], f32, tag="qkv")
                        for k in range(8):
                            nc.tensor.matmul(psq, wqg[:, k, :], hqT[:, k, sl],
                                             start=(k == 0), stop=(k == 7))
                        nc.vector.tensor_copy(qt[:, sl], psq)
                    # V natural per head, with ones column (col 64)
                    vxa = kvp.tile([128, 16, 65], f32r, tag="vxa")
                    vxb = kvp.tile([128, 16, 65], f32r, tag="vxb")
                    nc.vector.tensor_copy(vxa[:, :, 64:65], ones_f.bitcast(f32r))
                    nc.vector.tensor_copy(vxb[:, :, 64:65], ones_f.bitcast(f32r))
                    for kt_i in range(16):
                        pst = ps_tr.tile([128, 128], f32r, tag="tr")
                        nc.tensor.transpose(pst, vt[:, kt_i * 128:(kt_i + 1) * 128],
                                            ident_r)
                        nc.vector.tensor_copy(vxa[:, kt_i, 0:64], pst[:, 0:64])
                        nc.vector.tensor_copy(vxb[:, kt_i, 0:64], pst[:, 64:128])
                    dens = []
                    for h in range(2):
                        den_t = dnp.tile([1, TC], f32, tag=f"den{h}")
                        dens.append(den_t)
                    for j in range(NJ):
                        nkb = 4 * j + 4
                        qsl = bass.ds(j * QB, QB)
                        accs = []
                        for h in range(2):
                            acc_t = ps_acc.tile([65, QB], f32, tag=f"acc{h}")
                            accs.append(acc_t)
                        for kb in range(nkb):
                            rel = kb - 4 * j
                            for h in range(2):
                                hs = bass.ds(h * 64, 64)
                                st = ps_st.tile([128, QB], f32, tag="st")
                                nc.tensor.matmul(
                                    st, kt[hs, kb * 128:(kb + 1) * 128],
                                    qt[hs, qsl], start=True, stop=True,
                                    tile_position=(h * 64, 0))
                                pt = atw.tile([128, QB], f32r, tag="pt")
                                if rel < 0:
                                    nc.scalar.activation(out=pt, in_=st,
                                                         func=AF.Exp, scale=SCALE)
                                else:
                                    ptm = atw.tile([128, QB], f32, tag="ptm")
                                    nc.scalar.activation(out=ptm, in_=st,
                                                         func=AF.Exp, scale=SCALE)
                                    nc.gpsimd.tensor_mul(
                                        out=pt, in0=ptm, in1=mask_t[:, rel, :])
                                vx = vxa if h == 0 else vxb
                                nc.tensor.matmul(accs[h], vx[:, kb, :], pt,
                                                 start=(kb == 0),
                                                 stop=(kb == nkb - 1))
                        for h in range(2):
                            hs = bass.ds(h * 64, 64)
                            nc.scalar.copy(attT[hs, g, qsl], accs[h][0:64, :])
                            nc.scalar.copy(dens[h][0:1, qsl], accs[h][64:65, :])
                    for h in range(2):
                        hs = bass.ds(h * 64, 64)
                        # lane-parallel reciprocal: bounce [1,TC] through DRAM
                        # to [128, TC//128], recip+newton, back, broadcast-load
                        nc.sync.dma_start(out=den_dram[h, :], in_=dens[h][0:1, :])
                        dd = dnp.tile([128, TC // 128], f32, tag="dd")
                        nc.sync.dma_start(
                            out=dd,
                            in_=den_dram[h, :].rearrange("(p i) -> p i", p=128))
                        rr = dnp.tile([128, TC // 128], f32, tag="rr")
                        nc.vector.reciprocal(rr, dd)
                        nt2 = dnp.tile([128, TC // 128], f32, tag="nt2")
                        nc.vector.tensor_tensor(out=nt2, in0=dd, in1=rr,
                                                op=ALU.mult)
                        nc.vector.tensor_scalar(out=nt2, in0=nt2, scalar1=-1.0,
                                                scalar2=2.0, op0=ALU.mult,
                                                op1=ALU.add)
                        nc.vector.tensor_tensor(out=rr, in0=rr, in1=nt2,
                                                op=ALU.mult)
                        nc.sync.dma_start(
                            out=rden_dram[h, :].rearrange("(p i) -> p i", p=128),
                            in_=rr)
                        rb = dnp.tile([128, TC], f32, tag="rb")
                        nc.sync.dma_start(out=rb,
                                          in_=bcast_ap(rden_dram[h, :], 128, TC))
                        nc.vector.tensor_tensor(out=attT[hs, g, :],
                                                in0=attT[hs, g, :].bitcast(f32),
                                                in1=rb[hs, :], op=ALU.mult)

            # ---------- proj + residual -> x2 (spilled to DRAM) ----------
            with tc.tile_pool(name="proj_pool", bufs=2) as prp, \
                 tc.tile_pool(name="proj_c", bufs=1) as prc, \
                 tc.tile_pool(name="ps_proj", bufs=2, space="PSUM") as ps_proj:
                bp_b = prc.tile([128, D], f32)
                nc.sync.dma_start(out=bp_b, in_=bcast_ap(bp_d[:], 128, D))
                wp_sb = prc.tile([128, 8, D], f32r)
                nc.sync.dma_start(
                    out=wp_sb,
                    in_=wp_d[:, :].rearrange("(k p) o -> p k o", p=128).bitcast(f32r))
                for mt in range(8):
                    xqt = prp.tile([128, D], f32, tag="xq")
                    nc.sync.dma_start(out=xqt, in_=xq_d[mt * 128:(mt + 1) * 128, :])
                    for oc in range(2):
                        osl = bass.ds(oc * 512, 512)
                        ps = ps_proj.tile([128, 512], f32, tag="proj")
                        for k in range(8):
                            nc.tensor.matmul(ps, attT[:, k, mt * 128:(mt + 1) * 128],
                                             wp_sb[:, k, osl],
                                             start=(k == 0), stop=(k == 7))
                        tt = prp.tile([128, 512], f32, tag="tt")
                        nc.vector.tensor_tensor(out=tt, in0=ps, in1=xqt[:, osl],
                                                op=ALU.add)
                        nc.vector.tensor_tensor(out=tt, in0=tt, in1=bp_b[:, osl],
                                                op=ALU.add)
                        nc.sync.dma_start(
                            out=x2_dram[mt * 128:(mt + 1) * 128,
                                        oc * 512:(oc + 1) * 512],
                            in_=tt)

            # ---------- LN2 (transposed) + FFN per 512-token chunk ----------
            with tc.tile_pool(name="ffn_c", bufs=1) as fcc, \
                 tc.tile_pool(name="ffn_x", bufs=2) as fx, \
                 tc.tile_pool(name="ffn_w", bufs=3) as fw, \
                 tc.tile_pool(name="ln2_work", bufs=3) as lnw2, \
                 tc.tile_pool(name="ln2_stats", bufs=1) as lns2, \
                 tc.tile_pool(name="ps_ln2", bufs=1, space="PSUM") as ps_ln2, \
                 tc.tile_pool(name="ps_u", bufs=2, space="PSUM") as ps_u, \
                 tc.tile_pool(name="ps_v", bufs=1, space="PSUM") as ps_v:
                b1t = fcc.tile([128, 32], f32)
                nc.sync.dma_start(out=b1t, in_=b1_d[:, :].rearrange("i p -> p i"))
                b2_b = fcc.tile([128, D], f32)
                nc.sync.dma_start(out=b2_b, in_=bcast_ap(b2_d[:], 128, D))
                for tcx in range(2):
                    x2T = persist.tile([128, 8, 512], f32r, tag="t32a")
                    for mtl in range(4):
                        x2ld = fx.tile([128, D], f32, tag="x2ld")
                        mt = tcx * 4 + mtl
                        nc.sync.dma_start(out=x2ld,
                                          in_=x2_dram[mt * 128:(mt + 1) * 128, :])
                        for i in range(8):
                            pst = ps_u.tile([128, 128], f32, tag="u")
                            nc.tensor.transpose(pst, x2ld[:, i * 128:(i + 1) * 128],
                                                ident_f)
                            nc.vector.tensor_copy(
                                x2T[:, i, mtl * 128:(mtl + 1) * 128], pst)
                    ln_T(x2T, x2T, 512, g2t, be2t, lnw2, lns2, ps_ln2)
                    uT = persist.tile([128, 32, 512], f32r, tag="t64")
                    for i in range(32):
                        w1t = fw.tile([128, 8, 128], f32r, tag="w1t")
                        nc.sync.dma_start(
                            out=w1t,
                            in_=w1_d[i].rearrange("k p c -> p k c").bitcast(f32r))
                        psu = ps_u.tile([128, 512], f32, tag="u")
                        for k in range(8):
                            nc.tensor.matmul(psu, w1t[:, k, :], x2T[:, k, :],
                                             start=(k == 0), stop=(k == 7))
                        nc.scalar.activation(out=uT[:, i, :], in_=psu, func=AF.Relu,
                                             bias=b1t[:, i:i + 1])
                    for oc in range(2):
                        osl = bass.ds(oc * 512, 512)
                        psv = []
                        for mtl in range(4):
                            psv_t = ps_v.tile([128, 512], f32, tag=f"v{mtl}")
                            psv.append(psv_t)
                        for i in range(32):
                            w2t = fw.tile([128, 512], f32r, tag="w2t")
                            nc.sync.dma_start(
                                out=w2t,
                                in_=w2_d[i * 128:(i + 1) * 128,
                                         oc * 512:(oc + 1) * 512].bitcast(f32r))
                            for mtl in range(4):
                                nc.tensor.matmul(
                                    psv[mtl], uT[:, i, mtl * 128:(mtl + 1) * 128],
                                    w2t, start=(i == 0), stop=(i == 31))
                        for mtl in range(4):
                            mt = tcx * 4 + mtl
                            x2r = fx.tile([128, 512], f32, tag="x2r")
                            nc.sync.dma_start(
                                out=x2r,
                                in_=x2_dram[mt * 128:(mt + 1) * 128,
                                            oc * 512:(oc + 1) * 512])
                            ot = fx.tile([128, 512], f32, tag="ot")
                            nc.vector.tensor_tensor(out=ot, in0=psv[mtl],
                                                    in1=b2_b[:, osl], op=ALU.add)
                            nc.vector.tensor_tensor(out=ot, in0=ot, in1=x2r,
                                                    op=ALU.add)
                            nc.sync.dma_start(
                                out=out_d[mt * 128:(mt + 1) * 128,
                                          oc * 512:(oc + 1) * 512],
                                in_=ot)

    nc.compile()
    return nc


def _prep_shared(wq, wk, wv, wp, bp, w1, b1, w2, b2, g1, be1, g2, be2):
    c = np.ascontiguousarray
    f = np.float32

    def cf(a):
        return c(np.asarray(a, f))

    return {
        "wqp": c(np.asarray(wq, f).reshape(8, 128, 8, 128).transpose(2, 0, 1, 3)),
        "wkp": c(np.asarray(wk, f).reshape(8, 128, 8, 128).transpose(2, 0, 1, 3)),
        "wvp": c(np.asarray(wv, f).reshape(8, 128, 8, 128).transpose(2, 0, 1, 3)),
        "wp": cf(wp),
        "w1p": c(np.asarray(w1, f).reshape(8, 128, 32, 128).transpose(2, 0, 1, 3)),
        "w2": cf(w2),
        "g1t": cf(g1).reshape(8, 128),
        "be1t": cf(be1).reshape(8, 128),
        "g2t": cf(g2).reshape(8, 128),
        "be2t": cf(be2).reshape(8, 128),
        "b1t": cf(b1).reshape(32, 128),
        "bp": cf(bp),
        "b2": cf(b2),
    }


def _own_idx(p):
    return (np.arange(NJ)[:, None] * 512 + p * QB + np.arange(QB)[None, :]).ravel()


def _masks(p):
    m = np.zeros((4, 128, QB), np.float32)
    k = np.arange(128)[:, None]
    q = np.arange(QB)[None, :]
    for rel in range(4):
        m[rel] = (128 * rel + k <= QB * p + q).astype(np.float32)
    return m


def _make_in_maps(x, shared):
    in_maps = []
    for c in range(N_CORES):
        b, p = c // 2, c % 2
        xb = np.asarray(x[b], np.float32)
        idx = _own_idx(p)
        xq = np.ascontiguousarray(xb[idx])
        m = dict(shared)
        m["xT"] = np.ascontiguousarray(xb.T)
        m["xq"] = xq
        m["xqT"] = np.ascontiguousarray(xq.T)
        m["masks"] = _masks(p)
        in_maps.append(m)
    return in_maps


def kernel(**inputs):
    from concourse.bass_utils import run_bass_kernel_spmd

    if "nc" not in _cache:
        _cache["nc"] = _build()
    nc = _cache["nc"]

    shared = _prep_shared(
        inputs["wq"], inputs["wk"], inputs["wv"], inputs["wp"], inputs["bp"],
        inputs["w1"], inputs["b1"], inputs["w2"], inputs["b2"],
        inputs["g1"], inputs["be1"], inputs["g2"], inputs["be2"])
    in_maps = _make_in_maps(inputs["x"], shared)

    res = run_bass_kernel_spmd(nc, in_maps, list(range(N_CORES)))
    out = np.empty((B, T, D), np.float32)
    for c in range(N_CORES):
        b, p = c // 2, c % 2
        out[b][_own_idx(p)] = res.results[c]["out"]
    return out


# revision 12
# speedup vs baseline: 1.0900x; 1.0900x over previous
"""Trainium2 Bass kernel for a dense transformer block:
x -> LN1 -> causal MHA (16 heads) -> +residual -> LN2 -> FFN(4x, relu) -> +residual

Full inputs in, full outputs out. Sharding: 8 cores = (batch b in 0..3) x (parity p in 0..1).
Core (b, p) owns query 512-blocks {2j+p : j in 0..1} of batch b (1024 tokens), computes K/V
for the whole batch (duplicated within the pair), runs block-causal attention with a uniform
SPMD program (per-core causal masks and exp-bias passed as data), then proj/LN2/FFN on its
own token rows. No collectives. Matmuls in float32r (TF32-like, 1 cyc/row at free-dim>=256).

Tricks:
 - LN gains/biases folded into weights host-side (wq' = g1*wq, qbias = wq^T be1, ...), so the
   transposed-layout LN apply is 2 DVE passes and Q/K/V get biases via the PSUM->SBUF copy.
 - Reciprocals (LN rstd, softmax denominator) are single-partition vectors; DVE reciprocal is
   ~6 cyc/elem/lane, so [1,N] is bounced through DRAM into [128,N/128], inverted lane-parallel,
   bounced back, and broadcast-loaded via a 0-stride DMA.
 - Fully-masked causal blocks are zeroed by a -30000 per-partition bias into exp (free);
   diagonal blocks get an elementwise 0/1 mask multiply (split between DVE and GpSimd).
 - x2 (post-attention residual) spills to DRAM; FFN re-reads it.
"""

import numpy as np

B, T, D = 4, 2048, 1024
H, DH = 16, 64
NG = 8            # head groups of 2 heads
TC = 1024         # tokens per core
QB = 512          # query block
NJ = 2            # local query blocks per core
F4 = 4096
EPS = 1e-5
SCALE = float(D) ** -0.5
N_CORES = 8

_cache = {}


def _build():
    import contextlib
    import concourse.bass as bass
    import concourse.mybir as mybir
    import concourse.tile as tile
    from concourse import bacc
    from concourse.masks import make_identity

    f32, f32r = mybir.dt.float32, mybir.dt.float32r
    AF = mybir.ActivationFunctionType
    ALU = mybir.AluOpType

    nc = bacc.Bacc('TRN2', target_bir_lowering=False, debug=False,
                   num_devices=N_CORES)

    # ---- external I/O (per-core) ----
    xT_d = nc.dram_tensor("xT", [D, T], f32, kind="ExternalInput")
    xqT_d = nc.dram_tensor("xqT", [D, TC], f32, kind="ExternalInput")
    xq_d = nc.dram_tensor("xq", [TC, D], f32, kind="ExternalInput")
    wq_d = nc.dram_tensor("wqp", [NG, 8, 128, 128], f32, kind="ExternalInput")
    wk_d = nc.dram_tensor("wkp", [NG, 8, 128, 128], f32, kind="ExternalInput")
    wv_d = nc.dram_tensor("wvp", [NG, 8, 128, 128], f32, kind="ExternalInput")
    qb_d = nc.dram_tensor("qbias", [NG, 128], f32, kind="ExternalInput")
    kb_d = nc.dram_tensor("kbias", [NG, 128], f32, kind="ExternalInput")
    vb_d = nc.dram_tensor("vbias", [NG, 128], f32, kind="ExternalInput")
    wp_d = nc.dram_tensor("wp", [D, D], f32, kind="ExternalInput")
    w1_d = nc.dram_tensor("w1p", [32, 8, 128, 128], f32, kind="ExternalInput")
    w2_d = nc.dram_tensor("w2", [F4, D], f32, kind="ExternalInput")
    b1_d = nc.dram_tensor("b1t", [32, 128], f32, kind="ExternalInput")
    bp_d = nc.dram_tensor("bp", [D], f32, kind="ExternalInput")
    b2_d = nc.dram_tensor("b2", [D], f32, kind="ExternalInput")
    mk_d = nc.dram_tensor("masks", [8, 128, QB], f32, kind="ExternalInput")
    eb_d = nc.dram_tensor("expbias", [8], f32, kind="ExternalInput")
    out_d = nc.dram_tensor("out", [TC, D], f32, kind="ExternalOutput")

    x2_dram = nc.dram_tensor("x2_scratch", [TC, D], f32)
    den_dram = nc.dram_tensor("den_scratch", [2, TC], f32)
    rden_dram = nc.dram_tensor("rden_scratch", [2, TC], f32)
    mu_dram = nc.dram_tensor("mu_scratch", [2, 512], f32)
    sd_dram = nc.dram_tensor("sd_scratch", [2, 512], f32)
    rs_dram = nc.dram_tensor("rs_scratch", [2, 512], f32)

    def bcast_ap(dram_ap, parts, n):
        return bass.AP(tensor=dram_ap.tensor, offset=dram_ap.offset,
                       ap=[[0, parts], [1, n]])

    with tile.TileContext(nc) as tc:
        ctx = contextlib.ExitStack()
        with ctx:
            consts = ctx.enter_context(tc.tile_pool(name="consts", bufs=1))
            persist = ctx.enter_context(tc.tile_pool(name="persist", bufs=1))

            # ---------- constants ----------
            ident_f = consts.tile([128, 128], f32)
            make_identity(nc, ident_f)
            ident_r = consts.tile([128, 128], f32r)
            nc.vector.tensor_copy(ident_r, ident_f)
            ones_f = consts.tile([128, 16], f32)
            nc.vector.memset(ones_f, 1.0)
            ones_r = consts.tile([128, 1], f32r)
            nc.vector.tensor_copy(ones_r, ones_f[:, 0:1])
            eps_t = consts.tile([1, 1], f32)
            nc.vector.memset(eps_t, EPS)
            ebias_t = consts.tile([128, 8], f32)
            nc.sync.dma_start(out=ebias_t, in_=bcast_ap(eb_d[:], 128, 8))

            # ---------- transposed layernorm (in-place, 2-pass apply) ----------
            def ln_T(src_all, n_tok, wpool, spool, pspool):
                nch = n_tok // 512
                for c in range(nch):
                    sl = bass.ds(c * 512, 512)
                    cb = c % 2
                    mu_ps = pspool.tile([1, 512], f32, tag="mu_ps")
                    sq_ps = pspool.tile([1, 512], f32, tag="sq_ps")
                    for i in range(8):
                        sq = wpool.tile([128, 512], f32r, tag="sq")
                        nc.scalar.activation(out=sq, in_=src_all[:, i, sl].bitcast(f32),
                                             func=AF.Square)
                        nc.tensor.matmul(mu_ps, ones_r, src_all[:, i, sl],
                                         start=(i == 0), stop=(i == 7))
                        nc.tensor.matmul(sq_ps, ones_r, sq,
                                         start=(i == 0), stop=(i == 7))
                    mu = spool.tile([1, 512], f32, tag="mu")
                    nc.scalar.mul(mu, mu_ps, 1.0 / D)
                    sb = spool.tile([1, 512], f32, tag="sb")
                    nc.scalar.mul(sb, sq_ps, 1.0 / D)
                    sc = spool.tile([1, 512], f32, tag="sc")
                    nc.vector.tensor_tensor(out=sc, in0=mu, in1=mu, op=ALU.mult)
                    nc.vector.tensor_tensor(out=sb, in0=sb, in1=sc, op=ALU.subtract)
                    nc.scalar.activation(out=sb, in_=sb, func=AF.Sqrt, bias=eps_t)
                    # lane-parallel reciprocal via DRAM bounce
                    nc.sync.dma_start(out=mu_dram[cb, :], in_=mu)
                    nc.sync.dma_start(out=sd_dram[cb, :], in_=sb)
                    dd = spool.tile([128, 4], f32, tag="dd")
                    nc.sync.dma_start(
                        out=dd, in_=sd_dram[cb, :].rearrange("(p i) -> p i", p=128))
                    rr = spool.tile([128, 4], f32, tag="rr")
                    nc.vector.reciprocal(rr, dd)
                    nt = spool.tile([128, 4], f32, tag="nt")
                    nc.vector.tensor_tensor(out=nt, in0=dd, in1=rr, op=ALU.mult)
                    nc.vector.tensor_scalar(out=nt, in0=nt, scalar1=-1.0,
                                            scalar2=2.0, op0=ALU.mult, op1=ALU.add)
                    nc.vector.tensor_tensor(out=rr, in0=rr, in1=nt, op=ALU.mult)
                    nc.sync.dma_start(
                        out=rs_dram[cb, :].rearrange("(p i) -> p i", p=128), in_=rr)
                    mu_b = wpool.tile([128, 512], f32, tag="mu_b")
                    nc.sync.dma_start(out=mu_b, in_=bcast_ap(mu_dram[cb, :], 128, 512))
                    rstd_b = wpool.tile([128, 512], f32, tag="rstd_b")
                    nc.sync.dma_start(out=rstd_b,
                                      in_=bcast_ap(rs_dram[cb, :], 128, 512))
                    for i in range(8):
                        t1 = wpool.tile([128, 512], f32, tag="t1")
                        nc.vector.tensor_tensor(out=t1,
                                                in0=src_all[:, i, sl].bitcast(f32),
                                                in1=mu_b, op=ALU.subtract)
                        nc.vector.tensor_tensor(out=src_all[:, i, sl], in0=t1,
                                                in1=rstd_b, op=ALU.mult)

            # ---------- LN1: xqT first (small), then xT ----------
            # persist tags: t64: hT -> uT ; t32a: hqT -> x2T ; t32b: attT
            hqT = persist.tile([128, 8, TC], f32r, tag="t32a")
            hT = persist.tile([128, 8, T], f32r, tag="t64")
            with tc.tile_pool(name="ln_work", bufs=3) as lnw, \
                 tc.tile_pool(name="ln_stats", bufs=2) as lns, \
                 tc.tile_pool(name="ps_ln1", bufs=1, space="PSUM") as ps_ln1:
                for i in range(8):
                    nc.sync.dma_start(
                        out=hqT[:, i, :],
                        in_=xqT_d[i * 128:(i + 1) * 128, :].bitcast(f32r))
                ln_T(hqT, TC, lnw, lns, ps_ln1)
                for i in range(8):
                    nc.sync.dma_start(
                        out=hT[:, i, :],
                        in_=xT_d[i * 128:(i + 1) * 128, :].bitcast(f32r))
                ln_T(hT, T, lnw, lns, ps_ln1)

            # ---------- attention ----------
            attT = persist.tile([128, 8, TC], f32r, tag="t32b")
            with tc.tile_pool(name="kv_pool", bufs=1) as kvp, \
                 tc.tile_pool(name="wg_pool", bufs=1) as wgp, \
                 tc.tile_pool(name="att_work", bufs=3) as atw, \
                 tc.tile_pool(name="den_pool", bufs=1) as dnp, \
                 tc.tile_pool(name="ps_qkv", bufs=2, space="PSUM") as ps_qkv, \
                 tc.tile_pool(name="ps_st", bufs=3, space="PSUM") as ps_st, \
                 tc.tile_pool(name="ps_tr", bufs=1, space="PSUM") as ps_tr, \
                 tc.tile_pool(name="ps_acc", bufs=1, space="PSUM") as ps_acc, \
                 tc.tile_pool(name="att_c", bufs=1) as attc:
                mask_t = attc.tile([128, 8, QB], f32)
                nc.sync.dma_start(out=mask_t,
                                  in_=mk_d[:, :, :].rearrange("r p q -> p r q"))
                qbias_t = attc.tile([128, 8], f32)
                nc.sync.dma_start(out=qbias_t, in_=qb_d[:, :].rearrange("g p -> p g"))
                kbias_t = attc.tile([128, 8], f32)
                nc.sync.dma_start(out=kbias_t, in_=kb_d[:, :].rearrange("g p -> p g"))
                vbias_t = attc.tile([128, 8], f32)
                nc.sync.dma_start(out=vbias_t, in_=vb_d[:, :].rearrange("g p -> p g"))
                for g in range(NG):
                    wqg = wgp.tile([128, 8, 128], f32r, tag="wqg")
                    nc.sync.dma_start(
                        out=wqg, in_=wq_d[g].rearrange("k p c -> p k c").bitcast(f32r))
                    wkg = wgp.tile([128, 8, 128], f32r, tag="wkg")
                    nc.sync.dma_start(
                        out=wkg, in_=wk_d[g].rearrange("k p c -> p k c").bitcast(f32r))
                    wvg = wgp.tile([128, 8, 128], f32r, tag="wvg")
                    nc.sync.dma_start(
                        out=wvg, in_=wv_d[g].rearrange("k p c -> p k c").bitcast(f32r))
                    kt = kvp.tile([128, T], f32r, tag="kt")
                    vt = kvp.tile([128, T], f32r, tag="vt")
                    qt = kvp.tile([128, TC], f32r, tag="qt")
                    for n in range(4):
                        sl = bass.ds(n * 512, 512)
                        psk = ps_qkv.tile([128, 512], f32, tag="qkv")
                        for k in range(8):
                            nc.tensor.matmul(psk, wkg[:, k, :], hT[:, k, sl],
                                             start=(k == 0), stop=(k == 7))
                        nc.vector.tensor_scalar_add(kt[:, sl], psk,
                                                    kbias_t[:, g:g + 1])
                        psv = ps_qkv.tile([128, 512], f32, tag="qkv")
                        for k in range(8):
                            nc.tensor.matmul(psv, wvg[:, k, :], hT[:, k, sl],
                                             start=(k == 0), stop=(k == 7))
                        nc.vector.tensor_scalar_add(vt[:, sl], psv,
                                                    vbias_t[:, g:g + 1])
                    for n in range(2):
                        sl = bass.ds(n * 512, 512)
                        psq = ps_qkv.tile([128, 512], f32, tag="qkv")
                        for k in range(8):
                            nc.tensor.matmul(psq, wqg[:, k, :], hqT[:, k, sl],
                                             start=(k == 0), stop=(k == 7))
                        nc.vector.tensor_scalar_add(qt[:, sl], psq,
                                                    qbias_t[:, g:g + 1])
                    # V natural per head, with ones column (col 64)
                    vxa = kvp.tile([128, 16, 65], f32r, tag="vxa")
                    vxb = kvp.tile([128, 16, 65], f32r, tag="vxb")
                    nc.vector.tensor_copy(vxa[:, :, 64:65], ones_f.bitcast(f32r))
                    nc.vector.tensor_copy(vxb[:, :, 64:65], ones_f.bitcast(f32r))
                    for kt_i in range(16):
                        pst = ps_tr.tile([128, 128], f32r, tag="tr")
                        nc.tensor.transpose(pst, vt[:, kt_i * 128:(kt_i + 1) * 128],
                                            ident_r)
                        nc.vector.tensor_copy(vxa[:, kt_i, 0:64], pst[:, 0:64])
                        nc.vector.tensor_copy(vxb[:, kt_i, 0:64], pst[:, 64:128])
                    dens = []
                    for h in range(2):
                        den_t = dnp.tile([1, TC], f32, tag=f"den{h}")
                        dens.append(den_t)
                    for j in range(NJ):
                        nkb = 8 * j + 8
                        qsl = bass.ds(j * QB, QB)
                        accs = []
                        for h in range(2):
                            acc_t = ps_acc.tile([65, QB], f32, tag=f"acc{h}")
                            accs.append(acc_t)
                        for kb in range(nkb):
                            rel = kb - 8 * j
                            for h in range(2):
                                hs = bass.ds(h * 64, 64)
                                st = ps_st.tile([128, QB], f32, tag="st")
                                nc.tensor.matmul(
                                    st, kt[hs, kb * 128:(kb + 1) * 128],
                                    qt[hs, qsl], start=True, stop=True,
                                    tile_position=(h * 64, 0))
                                pt = atw.tile([128, QB], f32r, tag="pt")
                                if rel < 0:
                                    nc.scalar.activation(out=pt, in_=st,
                                                         func=AF.Exp, scale=SCALE)
                                else:
                                    nc.scalar.activation(
                                        out=pt, in_=st, func=AF.Exp, scale=SCALE,
                                        bias=ebias_t[:, rel:rel + 1])
                                    if rel < 4:
                                        nc.vector.tensor_tensor(
                                            out=pt, in0=pt.bitcast(f32),
                                            in1=mask_t[:, rel, :], op=ALU.mult)
                                    else:
                                        nc.gpsimd.tensor_mul(
                                            out=pt, in0=pt.bitcast(f32),
                                            in1=mask_t[:, rel, :])
                                vx = vxa if h == 0 else vxb
                                nc.tensor.matmul(accs[h], vx[:, kb, :], pt,
                                                 start=(kb == 0),
                                                 stop=(kb == nkb - 1))
                        for h in range(2):
                            hs = bass.ds(h * 64, 64)
                            nc.scalar.copy(attT[hs, g, qsl], accs[h][0:64, :])
                            nc.scalar.copy(dens[h][0:1, qsl], accs[h][64:65, :])
                    for h in range(2):
                        hs = bass.ds(h * 64, 64)
                        nc.sync.dma_start(out=den_dram[h, :], in_=dens[h][0:1, :])
                        dd = dnp.tile([128, TC // 128], f32, tag="dd")
                        nc.sync.dma_start(
                            out=dd,
                            in_=den_dram[h, :].rearrange("(p i) -> p i", p=128))
                        rr = dnp.tile([128, TC // 128], f32, tag="rr")
                        nc.vector.reciprocal(rr, dd)
                        nt2 = dnp.tile([128, TC // 128], f32, tag="nt2")
                        nc.vector.tensor_tensor(out=nt2, in0=dd, in1=rr,
                                                op=ALU.mult)
                        nc.vector.tensor_scalar(out=nt2, in0=nt2, scalar1=-1.0,
                                                scalar2=2.0, op0=ALU.mult,
                                                op1=ALU.add)
                        nc.vector.tensor_tensor(out=rr, in0=rr, in1=nt2,
                                                op=ALU.mult)
                        nc.sync.dma_start(
                            out=rden_dram[h, :].rearrange("(p i) -> p i", p=128),
                            in_=rr)
                        rb = dnp.tile([128, TC], f32, tag="rb")
                        nc.sync.dma_start(out=rb,
                                          in_=bcast_ap(rden_dram[h, :], 128, TC))
                        nc.vector.tensor_tensor(out=attT[hs, g, :],
                                                in0=attT[hs, g, :].bitcast(f32),
                                                in1=rb[hs, :], op=ALU.mult)

            # ---------- proj + residual -> x2 (spilled to DRAM) ----------
            with tc.tile_pool(name="proj_pool", bufs=2) as prp, \
                 tc.tile_pool(name="proj_c", bufs=1) as prc, \
                 tc.tile_pool(name="ps_proj", bufs=2, space="PSUM") as ps_proj:
                bp_b = prc.tile([128, D], f32)
                nc.sync.dma_start(out=bp_b, in_=bcast_ap(bp_d[:], 128, D))
                wp_sb = prc.tile([128, 8, D], f32r)
                nc.sync.dma_start(
                    out=wp_sb,
                    in_=wp_d[:, :].rearrange("(k p) o -> p k o", p=128).bitcast(f32r))
                for mt in range(8):
                    xqt = prp.tile([128, D], f32, tag="xq")
                    nc.sync.dma_start(out=xqt, in_=xq_d[mt * 128:(mt + 1) * 128, :])
                    for oc in range(2):
                        osl = bass.ds(oc * 512, 512)
                        ps = ps_proj.tile([128, 512], f32, tag="proj")
                        for k in range(8):
                            nc.tensor.matmul(ps, attT[:, k, mt * 128:(mt + 1) * 128],
                                             wp_sb[:, k, osl],
                                             start=(k == 0), stop=(k == 7))
                        tt = prp.tile([128, 512], f32, tag="tt")
                        nc.vector.tensor_tensor(out=tt, in0=ps, in1=xqt[:, osl],
                                                op=ALU.add)
                        nc.vector.tensor_tensor(out=tt, in0=tt, in1=bp_b[:, osl],
                                                op=ALU.add)
                        nc.sync.dma_start(
                            out=x2_dram[mt * 128:(mt + 1) * 128,
                                        oc * 512:(oc + 1) * 512],
                            in_=tt)

            # ---------- LN2 (transposed) + FFN per 512-token chunk ----------
            with tc.tile_pool(name="ffn_c", bufs=1) as fcc, \
                 tc.tile_pool(name="ffn_x", bufs=2) as fx, \
                 tc.tile_pool(name="ffn_w", bufs=3) as fw, \
                 tc.tile_pool(name="ln2_work", bufs=2) as lnw2, \
                 tc.tile_pool(name="ln2_stats", bufs=1) as lns2, \
                 tc.tile_pool(name="ps_ln2", bufs=1, space="PSUM") as ps_ln2, \
                 tc.tile_pool(name="ps_u", bufs=2, space="PSUM") as ps_u, \
                 tc.tile_pool(name="ps_v", bufs=1, space="PSUM") as ps_v:
                b1t = fcc.tile([128, 32], f32)
                nc.sync.dma_start(out=b1t, in_=b1_d[:, :].rearrange("i p -> p i"))
                b2_b = fcc.tile([128, D], f32)
                nc.sync.dma_start(out=b2_b, in_=bcast_ap(b2_d[:], 128, D))
                for tcx in range(2):
                    x2T = persist.tile([128, 8, 512], f32r, tag="t32a")
                    for mtl in range(4):
                        x2ld = fx.tile([128, D], f32, tag="x2ld")
                        mt = tcx * 4 + mtl
                        nc.sync.dma_start(out=x2ld,
                                          in_=x2_dram[mt * 128:(mt + 1) * 128, :])
                        for i in range(8):
                            pst = ps_u.tile([128, 128], f32, tag="u")
                            nc.tensor.transpose(pst, x2ld[:, i * 128:(i + 1) * 128],
                                                ident_f)
                            nc.vector.tensor_copy(
                                x2T[:, i, mtl * 128:(mtl + 1) * 128], pst)
                    ln_T(x2T, 512, lnw2, lns2, ps_ln2)
                    uT = persist.tile([128, 32, 512], f32r, tag="t64")
                    for i in range(32):
                        w1t = fw.tile([128, 8, 128], f32r, tag="w1t")
                        nc.sync.dma_start(
                            out=w1t,
                            in_=w1_d[i].rearrange("k p c -> p k c").bitcast(f32r))
                        psu = ps_u.tile([128, 512], f32, tag="u")
                        for k in range(8):
                            nc.tensor.matmul(psu, w1t[:, k, :], x2T[:, k, :],
                                             start=(k == 0), stop=(k == 7))
                        nc.scalar.activation(out=uT[:, i, :], in_=psu, func=AF.Relu,
                                             bias=b1t[:, i:i + 1])
                    for oc in range(2):
                        osl = bass.ds(oc * 512, 512)
                        psv = []
                        for mtl in range(4):
                            psv_t = ps_v.tile([128, 512], f32, tag=f"v{mtl}")
                            psv.append(psv_t)
                        for i in range(32):
                            w2t = fw.tile([128, 512], f32r, tag="w2t")
                            nc.sync.dma_start(
                                out=w2t,
                                in_=w2_d[i * 128:(i + 1) * 128,
                                         oc * 512:(oc + 1) * 512].bitcast(f32r))
                            for mtl in range(4):
                                nc.tensor.matmul(
                                    psv[mtl], uT[:, i, mtl * 128:(mtl + 1) * 128],
                                    w2t, start=(i == 0), stop=(i == 31))
                        for mtl in range(4):
                            mt = tcx * 4 + mtl
                            x2r = fx.tile([128, 512], f32, tag="x2r")
                            nc.sync.dma_start(
                                out=x2r,
                                in_=x2_dram[mt * 128:(mt + 1) * 128,
                                            oc * 512:(oc + 1) * 512])
                            ot = fx.tile([128, 512], f32, tag="ot")
                            nc.vector.tensor_tensor(out=ot, in0=psv[mtl],
                                                    in1=b2_b[:, osl], op=ALU.add)
                            nc.vector.tensor_tensor(out=ot, in0=ot, in1=x2r,
                                                    op=ALU.add)
                            nc.sync.dma_start(
                                out=out_d[mt * 128:(mt + 1) * 128,
                                          oc * 512:(oc + 1) * 512],
                                in_=ot)

    nc.compile()
    return nc


def _prep_shared(wq, wk, wv, wp, bp, w1, b1, w2, b2, g1, be1, g2, be2):
    c = np.ascontiguousarray
    f = np.float32

    def cf(a):
        return c(np.asarray(a, f))

    g1 = cf(g1)
    be1 = cf(be1)
    g2 = cf(g2)
    be2 = cf(be2)
    wq0, wk0, wv0, w10 = cf(wq), cf(wk), cf(wv), cf(w1)
    # fold LN gains into weights; LN biases become post-projection biases
    wqs = wq0 * g1[:, None]
    wks = wk0 * g1[:, None]
    wvs = wv0 * g1[:, None]
    w1s = w10 * g2[:, None]
    qbias = (wq0.T @ be1).astype(f)
    kbias = (wk0.T @ be1).astype(f)
    vbias = (wv0.T @ be1).astype(f)
    b1n = (cf(b1) + w10.T @ be2).astype(f)
    return {
        "wqp": c(wqs.reshape(8, 128, 8, 128).transpose(2, 0, 1, 3)),
        "wkp": c(wks.reshape(8, 128, 8, 128).transpose(2, 0, 1, 3)),
        "wvp": c(wvs.reshape(8, 128, 8, 128).transpose(2, 0, 1, 3)),
        "qbias": c(qbias.reshape(NG, 128)),
        "kbias": c(kbias.reshape(NG, 128)),
        "vbias": c(vbias.reshape(NG, 128)),
        "wp": cf(wp),
        "w1p": c(w1s.reshape(8, 128, 32, 128).transpose(2, 0, 1, 3)),
        "w2": cf(w2),
        "b1t": c(b1n.reshape(32, 128)),
        "bp": cf(bp),
        "b2": cf(b2),
    }


def _own_idx(p):
    return (np.arange(NJ)[:, None] * 1024 + p * QB + np.arange(QB)[None, :]).ravel()


def _masks(p):
    m = np.zeros((8, 128, QB), np.float32)
    k = np.arange(128)[:, None]
    q = np.arange(QB)[None, :]
    for rel in range(8):
        m[rel] = (128 * rel + k <= QB * p + q).astype(np.float32)
    return m


def _expbias(p):
    e = np.zeros(8, np.float32)
    if p == 0:
        e[4:] = -30000.0
    return e


def _make_in_maps(x, shared):
    in_maps = []
    for c in range(N_CORES):
        b, p = c // 2, c % 2
        xb = np.asarray(x[b], np.float32)
        idx = _own_idx(p)
        xq = np.ascontiguousarray(xb[idx])
        m = dict(shared)
        m["xT"] = np.ascontiguousarray(xb.T)
        m["xq"] = xq
        m["xqT"] = np.ascontiguousarray(xq.T)
        m["masks"] = _masks(p)
        m["expbias"] = _expbias(p)
        in_maps.append(m)
    return in_maps


def kernel(**inputs):
    from concourse.bass_utils import run_bass_kernel_spmd

    if "nc" not in _cache:
        _cache["nc"] = _build()
    nc = _cache["nc"]

    shared = _prep_shared(
        inputs["wq"], inputs["wk"], inputs["wv"], inputs["wp"], inputs["bp"],
        inputs["w1"], inputs["b1"], inputs["w2"], inputs["b2"],
        inputs["g1"], inputs["be1"], inputs["g2"], inputs["be2"])
    in_maps = _make_in_maps(inputs["x"], shared)

    res = run_bass_kernel_spmd(nc, in_maps, list(range(N_CORES)))
    out = np.empty((B, T, D), np.float32)
    for c in range(N_CORES):
        b, p = c // 2, c % 2
        out[b][_own_idx(p)] = res.results[c]["out"]
    return out


# revision 16
# speedup vs baseline: 1.3953x; 1.2802x over previous
"""Trainium2 Bass kernel for a dense transformer block:
x -> LN1 -> causal MHA (16 heads) -> +residual -> LN2 -> FFN(4x, relu) -> +residual

Full inputs in, full outputs out. Sharding: 8 cores = (batch b in 0..3) x (parity p in 0..1).
Core (b, p) owns query 512-blocks {2j+p : j in 0..1} of batch b (1024 tokens), computes K/V
for the whole batch (duplicated within the pair), runs block-causal attention with a uniform
SPMD program (per-core causal masks and exp-bias passed as data), then proj/LN2/FFN on its
own token rows. No collectives. Matmuls in float32r (TF32-like, 1 cyc/row at free-dim>=256).

Tricks:
 - LN gains/biases folded into weights host-side (wq' = g1*wq, qbias = wq^T be1, ...), so the
   transposed-layout LN apply is 2 DVE passes and Q/K/V get biases via the PSUM->SBUF copy.
 - Reciprocals (LN rstd, softmax denominator) are single-partition vectors; DVE reciprocal is
   ~6 cyc/elem/lane, so [1,N] is bounced through DRAM into [128,N/128], inverted lane-parallel,
   bounced back, and broadcast-loaded via a 0-stride DMA.
 - Fully-masked causal blocks are zeroed by a -30000 per-partition bias into exp (free);
   diagonal blocks get an elementwise 0/1 mask multiply (split between DVE and GpSimd).
 - x2 (post-attention residual) spills to DRAM; FFN re-reads it.
"""

import numpy as np

B, T, D = 4, 2048, 1024
H, DH = 16, 64
NG = 8            # head groups of 2 heads
TC = 1024         # tokens per core
QB = 512          # query block
NJ = 2            # local query blocks per core
F4 = 4096
EPS = 1e-5
SCALE = float(D) ** -0.5
N_CORES = 8

_cache = {}


def _build():
    import contextlib
    import concourse.bass as bass
    import concourse.mybir as mybir
    import concourse.tile as tile
    from concourse import bacc
    from concourse.masks import make_identity

    f32, f32r = mybir.dt.float32, mybir.dt.float32r
    bf16 = mybir.dt.bfloat16
    AF = mybir.ActivationFunctionType
    ALU = mybir.AluOpType

    nc = bacc.Bacc('TRN2', target_bir_lowering=False, debug=False,
                   num_devices=N_CORES)

    # ---- external I/O (per-core) ----
    xT_d = nc.dram_tensor("xT", [D, T], f32, kind="ExternalInput")
    xqT_d = nc.dram_tensor("xqT", [D, TC], f32, kind="ExternalInput")
    xq_d = nc.dram_tensor("xq", [TC, D], f32, kind="ExternalInput")
    wq_d = nc.dram_tensor("wqp", [NG, 8, 128, 128], f32, kind="ExternalInput")
    wk_d = nc.dram_tensor("wkp", [NG, 8, 128, 128], f32, kind="ExternalInput")
    wv_d = nc.dram_tensor("wvp", [NG, 8, 128, 128], f32, kind="ExternalInput")
    qb_d = nc.dram_tensor("qbias", [NG, 128], f32, kind="ExternalInput")
    kb_d = nc.dram_tensor("kbias", [NG, 128], f32, kind="ExternalInput")
    vb_d = nc.dram_tensor("vbias", [NG, 128], f32, kind="ExternalInput")
    wp_d = nc.dram_tensor("wp", [D, D], f32, kind="ExternalInput")
    w1_d = nc.dram_tensor("w1p", [32, 8, 128, 128], f32, kind="ExternalInput")
    w2_d = nc.dram_tensor("w2", [F4, D], f32, kind="ExternalInput")
    b1_d = nc.dram_tensor("b1t", [32, 128], f32, kind="ExternalInput")
    bp_d = nc.dram_tensor("bp", [D], f32, kind="ExternalInput")
    b2_d = nc.dram_tensor("b2", [D], f32, kind="ExternalInput")
    mk_d = nc.dram_tensor("masks", [8, 128, QB], f32, kind="ExternalInput")
    eb_d = nc.dram_tensor("expbias", [8], f32, kind="ExternalInput")
    out_d = nc.dram_tensor("out", [TC, D], f32, kind="ExternalOutput")

    x2_dram = nc.dram_tensor("x2_scratch", [TC, D], f32)
    den_dram = nc.dram_tensor("den_scratch", [2, TC], f32)
    rden_dram = nc.dram_tensor("rden_scratch", [2, TC], f32)
    mu_dram = nc.dram_tensor("mu_scratch", [2, 512], f32)
    sd_dram = nc.dram_tensor("sd_scratch", [2, 512], f32)
    rs_dram = nc.dram_tensor("rs_scratch", [2, 512], f32)

    def bcast_ap(dram_ap, parts, n):
        return bass.AP(tensor=dram_ap.tensor, offset=dram_ap.offset,
                       ap=[[0, parts], [1, n]])

    with tile.TileContext(nc) as tc:
        ctx = contextlib.ExitStack()
        with ctx:
            consts = ctx.enter_context(tc.tile_pool(name="consts", bufs=1))
            persist = ctx.enter_context(tc.tile_pool(name="persist", bufs=1))

            # ---------- constants ----------
            ident_f = consts.tile([128, 128], f32)
            make_identity(nc, ident_f)
            ident_r = consts.tile([128, 128], f32r)
            nc.vector.tensor_copy(ident_r, ident_f)
            ones_f = consts.tile([128, 16], f32)
            nc.vector.memset(ones_f, 1.0)
            ones_r = consts.tile([128, 1], f32r)
            nc.vector.tensor_copy(ones_r, ones_f[:, 0:1])
            eps_t = consts.tile([1, 1], f32)
            nc.vector.memset(eps_t, EPS)
            ebias_t = consts.tile([128, 8], f32)
            nc.sync.dma_start(out=ebias_t, in_=bcast_ap(eb_d[:], 128, 8))

            # ---------- transposed layernorm (in-place, 2-pass apply) ----------
            def ln_T(src_all, n_tok, wpool, spool, pspool):
                nch = n_tok // 512
                for c in range(nch):
                    sl = bass.ds(c * 512, 512)
                    cb = c % 2
                    mu_ps = pspool.tile([1, 512], f32, tag="mu_ps")
                    sq_ps = pspool.tile([1, 512], f32, tag="sq_ps")
                    for i in range(8):
                        sq = wpool.tile([128, 512], f32r, tag="sq")
                        nc.scalar.activation(out=sq, in_=src_all[:, i, sl].bitcast(f32),
                                             func=AF.Square)
                        nc.tensor.matmul(mu_ps, ones_r, src_all[:, i, sl],
                                         start=(i == 0), stop=(i == 7))
                        nc.tensor.matmul(sq_ps, ones_r, sq,
                                         start=(i == 0), stop=(i == 7))
                    mu = spool.tile([1, 512], f32, tag="mu")
                    nc.scalar.mul(mu, mu_ps, 1.0 / D)
                    sb = spool.tile([1, 512], f32, tag="sb")
                    nc.scalar.mul(sb, sq_ps, 1.0 / D)
                    sc = spool.tile([1, 512], f32, tag="sc")
                    nc.vector.tensor_tensor(out=sc, in0=mu, in1=mu, op=ALU.mult)
                    nc.vector.tensor_tensor(out=sb, in0=sb, in1=sc, op=ALU.subtract)
                    nc.scalar.activation(out=sb, in_=sb, func=AF.Sqrt, bias=eps_t)
                    # lane-parallel reciprocal via DRAM bounce
                    nc.sync.dma_start(out=mu_dram[cb, :], in_=mu)
                    nc.sync.dma_start(out=sd_dram[cb, :], in_=sb)
                    dd = spool.tile([128, 4], f32, tag="dd")
                    nc.sync.dma_start(
                        out=dd, in_=sd_dram[cb, :].rearrange("(p i) -> p i", p=128))
                    rr = spool.tile([128, 4], f32, tag="rr")
                    nc.vector.reciprocal(rr, dd)
                    nt = spool.tile([128, 4], f32, tag="nt")
                    nc.vector.tensor_tensor(out=nt, in0=dd, in1=rr, op=ALU.mult)
                    nc.vector.tensor_scalar(out=nt, in0=nt, scalar1=-1.0,
                                            scalar2=2.0, op0=ALU.mult, op1=ALU.add)
                    nc.vector.tensor_tensor(out=rr, in0=rr, in1=nt, op=ALU.mult)
                    nc.sync.dma_start(
                        out=rs_dram[cb, :].rearrange("(p i) -> p i", p=128), in_=rr)
                    mu_b = wpool.tile([128, 512], f32, tag="mu_b")
                    nc.sync.dma_start(out=mu_b, in_=bcast_ap(mu_dram[cb, :], 128, 512))
                    rstd_b = wpool.tile([128, 512], f32, tag="rstd_b")
                    nc.sync.dma_start(out=rstd_b,
                                      in_=bcast_ap(rs_dram[cb, :], 128, 512))
                    for i in range(8):
                        t1 = wpool.tile([128, 512], f32, tag="t1")
                        nc.vector.tensor_tensor(out=t1,
                                                in0=src_all[:, i, sl].bitcast(f32),
                                                in1=mu_b, op=ALU.subtract)
                        nc.vector.tensor_tensor(out=src_all[:, i, sl], in0=t1,
                                                in1=rstd_b, op=ALU.mult)

            # ---------- LN1: xqT first (small), then xT ----------
            # persist tags: t64: hT -> uT ; t32a: hqT -> x2T ; t32b: attT
            hqT = persist.tile([128, 8, TC], f32r, tag="t32a")
            hT = persist.tile([128, 8, T], f32r, tag="t64")
            with tc.tile_pool(name="ln_work", bufs=3) as lnw, \
                 tc.tile_pool(name="ln_stats", bufs=2) as lns, \
                 tc.tile_pool(name="ps_ln1", bufs=1, space="PSUM") as ps_ln1:
                for i in range(8):
                    nc.sync.dma_start(
                        out=hqT[:, i, :],
                        in_=xqT_d[i * 128:(i + 1) * 128, :].bitcast(f32r))
                ln_T(hqT, TC, lnw, lns, ps_ln1)
                for i in range(8):
                    nc.sync.dma_start(
                        out=hT[:, i, :],
                        in_=xT_d[i * 128:(i + 1) * 128, :].bitcast(f32r))
                ln_T(hT, T, lnw, lns, ps_ln1)

            # ---------- attention ----------
            attT = persist.tile([128, 8, TC], f32r, tag="t32b")
            with tc.tile_pool(name="kv_pool", bufs=1) as kvp, \
                 tc.tile_pool(name="wg_pool", bufs=1) as wgp, \
                 tc.tile_pool(name="att_work", bufs=3) as atw, \
                 tc.tile_pool(name="den_pool", bufs=1) as dnp, \
                 tc.tile_pool(name="ps_qkv", bufs=2, space="PSUM") as ps_qkv, \
                 tc.tile_pool(name="ps_st", bufs=3, space="PSUM") as ps_st, \
                 tc.tile_pool(name="ps_acc", bufs=1, space="PSUM") as ps_acc, \
                 tc.tile_pool(name="att_c", bufs=1) as attc:
                mask_t = attc.tile([128, 8, QB], f32)
                nc.sync.dma_start(out=mask_t,
                                  in_=mk_d[:, :, :].rearrange("r p q -> p r q"))
                qbias_t = attc.tile([128, 8], f32)
                nc.sync.dma_start(out=qbias_t, in_=qb_d[:, :].rearrange("g p -> p g"))
                kbias_t = attc.tile([128, 8], f32)
                nc.sync.dma_start(out=kbias_t, in_=kb_d[:, :].rearrange("g p -> p g"))
                vbias_t = attc.tile([128, 8], f32)
                nc.sync.dma_start(out=vbias_t, in_=vb_d[:, :].rearrange("g p -> p g"))
                for g in range(NG):
                    wqg = wgp.tile([128, 8, 128], f32r, tag="wqg")
                    nc.sync.dma_start(
                        out=wqg, in_=wq_d[g].rearrange("k p c -> p k c").bitcast(f32r))
                    wkg = wgp.tile([128, 8, 128], f32r, tag="wkg")
                    nc.sync.dma_start(
                        out=wkg, in_=wk_d[g].rearrange("k p c -> p k c").bitcast(f32r))
                    wvg = wgp.tile([128, 8, 128], f32r, tag="wvg")
                    nc.sync.dma_start(
                        out=wvg, in_=wv_d[g].rearrange("k p c -> p k c").bitcast(f32r))
                    kt = kvp.tile([128, T], bf16, tag="kt")
                    vt = kvp.tile([128, T], f32r, tag="vt")
                    qt = kvp.tile([128, TC], bf16, tag="qt")
                    for n in range(4):
                        sl = bass.ds(n * 512, 512)
                        psk = ps_qkv.tile([128, 512], f32, tag="qkv")
                        for k in range(8):
                            nc.tensor.matmul(psk, wkg[:, k, :], hT[:, k, sl],
                                             start=(k == 0), stop=(k == 7))
                        nc.vector.tensor_scalar_add(kt[:, sl], psk,
                                                    kbias_t[:, g:g + 1])
                        psv = ps_qkv.tile([128, 512], f32, tag="qkv")
                        for k in range(8):
                            nc.tensor.matmul(psv, wvg[:, k, :], hT[:, k, sl],
                                             start=(k == 0), stop=(k == 7))
                        nc.vector.tensor_scalar_add(vt[:, sl], psv,
                                                    vbias_t[:, g:g + 1])
                    for n in range(2):
                        sl = bass.ds(n * 512, 512)
                        psq = ps_qkv.tile([128, 512], f32, tag="qkv")
                        for k in range(8):
                            nc.tensor.matmul(psq, wqg[:, k, :], hqT[:, k, sl],
                                             start=(k == 0), stop=(k == 7))
                        nc.vector.tensor_scalar_add(qt[:, sl], psq,
                                                    qbias_t[:, g:g + 1])
                    # V natural per head, with ones column (col 64)
                    vxa = kvp.tile([128, 16, 65], f32r, tag="vxa")
                    vxb = kvp.tile([128, 16, 65], f32r, tag="vxb")
                    nc.vector.tensor_copy(vxa[:, :, 64:65], ones_f.bitcast(f32r))
                    nc.vector.tensor_copy(vxb[:, :, 64:65], ones_f.bitcast(f32r))
                    for kt_i in range(16):
                        pst = ps_tr.tile([128, 128], f32r, tag="tr")
                        nc.tensor.transpose(pst, vt[:, kt_i * 128:(kt_i + 1) * 128],
                                            ident_r)
                        nc.vector.tensor_copy(vxa[:, kt_i, 0:64], pst[:, 0:64])
                        nc.vector.tensor_copy(vxb[:, kt_i, 0:64], pst[:, 64:128])
                    dens = []
                    for h in range(2):
                        den_t = dnp.tile([1, TC], f32, tag=f"den{h}")
                        dens.append(den_t)
                    for j in range(NJ):
                        nkb = 8 * j + 8
                        qsl = bass.ds(j * QB, QB)
                        accs = []
                        for h in range(2):
                            acc_t = ps_acc.tile([65, QB], f32, tag=f"acc{h}")
                            accs.append(acc_t)
                        for kb in range(nkb):
                            rel = kb - 8 * j
                            for h in range(2):
                                hs = bass.ds(h * 64, 64)
                                st = ps_st.tile([128, QB], f32, tag="st")
                                nc.tensor.matmul(
                                    st, kt[hs, kb * 128:(kb + 1) * 128],
                                    qt[hs, qsl], start=True, stop=True,
                                    tile_position=(h * 64, 0))
                                pt = atw.tile([128, QB], f32r, tag="pt")
                                if rel < 0:
                                    nc.scalar.activation(out=pt, in_=st,
                                                         func=AF.Exp, scale=SCALE)
                                else:
                                    nc.scalar.activation(
                                        out=pt, in_=st, func=AF.Exp, scale=SCALE,
                                        bias=ebias_t[:, rel:rel + 1])
                                    if rel < 4:
                                        nc.vector.tensor_tensor(
                                            out=pt, in0=pt.bitcast(f32),
                                            in1=mask_t[:, rel, :], op=ALU.mult)
                                    else:
                                        nc.gpsimd.tensor_mul(
                                            out=pt, in0=pt.bitcast(f32),
                                            in1=mask_t[:, rel, :])
                                vx = vxa if h == 0 else vxb
                                nc.tensor.matmul(accs[h], vx[:, kb, :], pt,
                                                 start=(kb == 0),
                                                 stop=(kb == nkb - 1))
                        for h in range(2):
                            hs = bass.ds(h * 64, 64)
                            nc.scalar.copy(attT[hs, g, qsl], accs[h][0:64, :])
                            nc.scalar.copy(dens[h][0:1, qsl], accs[h][64:65, :])
                    for h in range(2):
                        hs = bass.ds(h * 64, 64)
                        nc.sync.dma_start(out=den_dram[h, :], in_=dens[h][0:1, :])
                        dd = dnp.tile([128, TC // 128], f32, tag="dd")
                        nc.sync.dma_start(
                            out=dd,
                            in_=den_dram[h, :].rearrange("(p i) -> p i", p=128))
                        rr = dnp.tile([128, TC // 128], f32, tag="rr")
                        nc.vector.reciprocal(rr, dd)
                        nt2 = dnp.tile([128, TC // 128], f32, tag="nt2")
                        nc.vector.tensor_tensor(out=nt2, in0=dd, in1=rr,
                                                op=ALU.mult)
                        nc.vector.tensor_scalar(out=nt2, in0=nt2, scalar1=-1.0,
                                                scalar2=2.0, op0=ALU.mult,
                                                op1=ALU.add)
                        nc.vector.tensor_tensor(out=rr, in0=rr, in1=nt2,
                                                op=ALU.mult)
                        nc.sync.dma_start(
                            out=rden_dram[h, :].rearrange("(p i) -> p i", p=128),
                            in_=rr)
                        rb = dnp.tile([128, TC], f32, tag="rb")
                        nc.sync.dma_start(out=rb,
                                          in_=bcast_ap(rden_dram[h, :], 128, TC))
                        nc.vector.tensor_tensor(out=attT[hs, g, :],
                                                in0=attT[hs, g, :].bitcast(f32),
                                                in1=rb[hs, :], op=ALU.mult)

            # ---------- proj + residual -> x2 (spilled to DRAM) ----------
            with tc.tile_pool(name="proj_pool", bufs=2) as prp, \
                 tc.tile_pool(name="proj_c", bufs=1) as prc, \
                 tc.tile_pool(name="ps_proj", bufs=2, space="PSUM") as ps_proj:
                bp_b = prc.tile([128, D], f32)
                nc.sync.dma_start(out=bp_b, in_=bcast_ap(bp_d[:], 128, D))
                wp_sb = prc.tile([128, 8, D], f32r)
                nc.sync.dma_start(
                    out=wp_sb,
                    in_=wp_d[:, :].rearrange("(k p) o -> p k o", p=128).bitcast(f32r))
                for mt in range(8):
                    xqt = prp.tile([128, D], f32, tag="xq")
                    nc.sync.dma_start(out=xqt, in_=xq_d[mt * 128:(mt + 1) * 128, :])
                    for oc in range(2):
                        osl = bass.ds(oc * 512, 512)
                        ps = ps_proj.tile([128, 512], f32, tag="proj")
                        for k in range(8):
                            nc.tensor.matmul(ps, attT[:, k, mt * 128:(mt + 1) * 128],
                                             wp_sb[:, k, osl],
                                             start=(k == 0), stop=(k == 7))
                        tt = prp.tile([128, 512], f32, tag="tt")
                        nc.vector.tensor_tensor(out=tt, in0=ps, in1=xqt[:, osl],
                                                op=ALU.add)
                        nc.vector.tensor_tensor(out=tt, in0=tt, in1=bp_b[:, osl],
                                                op=ALU.add)
                        nc.sync.dma_start(
                            out=x2_dram[mt * 128:(mt + 1) * 128,
                                        oc * 512:(oc + 1) * 512],
                            in_=tt)

            # ---------- LN2 (transposed) + FFN per 512-token chunk ----------
            with tc.tile_pool(name="ffn_c", bufs=1) as fcc, \
                 tc.tile_pool(name="ffn_x", bufs=2) as fx, \
                 tc.tile_pool(name="ffn_w", bufs=3) as fw, \
                 tc.tile_pool(name="ln2_work", bufs=2) as lnw2, \
                 tc.tile_pool(name="ln2_stats", bufs=1) as lns2, \
                 tc.tile_pool(name="ps_ln2", bufs=1, space="PSUM") as ps_ln2, \
                 tc.tile_pool(name="ps_u", bufs=2, space="PSUM") as ps_u, \
                 tc.tile_pool(name="ps_v", bufs=1, space="PSUM") as ps_v:
                b1t = fcc.tile([128, 32], f32)
                nc.sync.dma_start(out=b1t, in_=b1_d[:, :].rearrange("i p -> p i"))
                b2_b = fcc.tile([128, D], f32)
                nc.sync.dma_start(out=b2_b, in_=bcast_ap(b2_d[:], 128, D))
                for tcx in range(2):
                    x2T = persist.tile([128, 8, 512], f32r, tag="t32a")
                    for mtl in range(4):
                        x2ld = fx.tile([128, D], f32, tag="x2ld")
                        mt = tcx * 4 + mtl
                        nc.sync.dma_start(out=x2ld,
                                          in_=x2_dram[mt * 128:(mt + 1) * 128, :])
                        for i in range(8):
                            pst = ps_u.tile([128, 128], f32, tag="u")
                            nc.tensor.transpose(pst, x2ld[:, i * 128:(i + 1) * 128],
                                                ident_f)
                            nc.vector.tensor_copy(
                                x2T[:, i, mtl * 128:(mtl + 1) * 128], pst)
                    ln_T(x2T, 512, lnw2, lns2, ps_ln2)
                    uT = persist.tile([128, 32, 512], f32r, tag="t64")
                    for i in range(32):
                        w1t = fw.tile([128, 8, 128], f32r, tag="w1t")
                        nc.sync.dma_start(
                            out=w1t,
                            in_=w1_d[i].rearrange("k p c -> p k c").bitcast(f32r))
                        psu = ps_u.tile([128, 512], f32, tag="u")
                        for k in range(8):
                            nc.tensor.matmul(psu, w1t[:, k, :], x2T[:, k, :],
                                             start=(k == 0), stop=(k == 7))
                        nc.scalar.activation(out=uT[:, i, :], in_=psu, func=AF.Relu,
                                             bias=b1t[:, i:i + 1])
                    for oc in range(2):
                        osl = bass.ds(oc * 512, 512)
                        psv = []
                        for mtl in range(4):
                            psv_t = ps_v.tile([128, 512], f32, tag=f"v{mtl}")
                            psv.append(psv_t)
                        for i in range(32):
                            w2t = fw.tile([128, 512], f32r, tag="w2t")
                            nc.sync.dma_start(
                                out=w2t,
                                in_=w2_d[i * 128:(i + 1) * 128,
                                         oc * 512:(oc + 1) * 512].bitcast(f32r))
                            for mtl in range(4):
                                nc.tensor.matmul(
                                    psv[mtl], uT[:, i, mtl * 128:(mtl + 1) * 128],
                                    w2t, start=(i == 0), stop=(i == 31))
                        for mtl in range(4):
                            mt = tcx * 4 + mtl
                            x2r = fx.tile([128, 512], f32, tag="x2r")
                            nc.sync.dma_start(
                                out=x2r,
                                in_=x2_dram[mt * 128:(mt + 1) * 128,
                                            oc * 512:(oc + 1) * 512])
                            ot = fx.tile([128, 512], f32, tag="ot")
                            nc.vector.tensor_tensor(out=ot, in0=psv[mtl],
                                                    in1=b2_b[:, osl], op=ALU.add)
                            nc.vector.tensor_tensor(out=ot, in0=ot, in1=x2r,
                                                    op=ALU.add)
                            nc.sync.dma_start(
                                out=out_d[mt * 128:(mt + 1) * 128,
                                          oc * 512:(oc + 1) * 512],
                                in_=ot)

    nc.compile()
    return nc


def _prep_shared(wq, wk, wv, wp, bp, w1, b1, w2, b2, g1, be1, g2, be2):
    c = np.ascontiguousarray
    f = np.float32

    def cf(a):
        return c(np.asarray(a, f))

    g1 = cf(g1)
    be1 = cf(be1)
    g2 = cf(g2)
    be2 = cf(be2)
    wq0, wk0, wv0, w10 = cf(wq), cf(wk), cf(wv), cf(w1)
    # fold LN gains into weights; LN biases become post-projection biases
    wqs = wq0 * g1[:, None]
    wks = wk0 * g1[:, None]
    wvs = wv0 * g1[:, None]
    w1s = w10 * g2[:, None]
    qbias = (wq0.T @ be1).astype(f)
    kbias = (wk0.T @ be1).astype(f)
    vbias = (wv0.T @ be1).astype(f)
    b1n = (cf(b1) + w10.T @ be2).astype(f)
    return {
        "wqp": c(wqs.reshape(8, 128, 8, 128).transpose(2, 0, 1, 3)),
        "wkp": c(wks.reshape(8, 128, 8, 128).transpose(2, 0, 1, 3)),
        "wvp": c(wvs.reshape(8, 128, 8, 128).transpose(2, 0, 1, 3)),
        "qbias": c(qbias.reshape(NG, 128)),
        "kbias": c(kbias.reshape(NG, 128)),
        "vbias": c(vbias.reshape(NG, 128)),
        "wp": cf(wp),
        "w1p": c(w1s.reshape(8, 128, 32, 128).transpose(2, 0, 1, 3)),
        "w2": cf(w2),
        "b1t": c(b1n.reshape(32, 128)),
        "bp": cf(bp),
        "b2": cf(b2),
    }


def _own_idx(p):
    return (np.arange(NJ)[:, None] * 1024 + p * QB + np.arange(QB)[None, :]).ravel()


def _masks(p):
    m = np.zeros((8, 128, QB), np.float32)
    k = np.arange(128)[:, None]
    q = np.arange(QB)[None, :]
    for rel in range(8):
        m[rel] = (128 * rel + k <= QB * p + q).astype(np.float32)
    return m


def _expbias(p):
    e = np.zeros(8, np.float32)
    if p == 0:
        e[4:] = -30000.0
    return e


def _make_in_maps(x, shared):
    in_maps = []
    for c in range(N_CORES):
        b, p = c // 2, c % 2
        xb = np.asarray(x[b], np.float32)
        idx = _own_idx(p)
        xq = np.ascontiguousarray(xb[idx])
        m = dict(shared)
        m["xT"] = np.ascontiguousarray(xb.T)
        m["xq"] = xq
        m["xqT"] = np.ascontiguousarray(xq.T)
        m["masks"] = _masks(p)
        m["expbias"] = _expbias(p)
        in_maps.append(m)
    return in_maps


def kernel(**inputs):
    from concourse.bass_utils import run_bass_kernel_spmd

    if "nc" not in _cache:
        _cache["nc"] = _build()
    nc = _cache["nc"]

    shared = _prep_shared(
        inputs["wq"], inputs["wk"], inputs["wv"], inputs["wp"], inputs["bp"],
        inputs["w1"], inputs["b1"], inputs["w2"], inputs["b2"],
        inputs["g1"], inputs["be1"], inputs["g2"], inputs["be2"])
    in_maps = _make_in_maps(inputs["x"], shared)

    res = run_bass_kernel_spmd(nc, in_maps, list(range(N_CORES)))
    out = np.empty((B, T, D), np.float32)
    for c in range(N_CORES):
        b, p = c // 2, c % 2
        out[b][_own_idx(p)] = res.results[c]["out"]
    return out


# revision 17
# speedup vs baseline: 1.4573x; 1.0444x over previous
"""Trainium2 Bass kernel for a dense transformer block:
x -> LN1 -> causal MHA (16 heads) -> +residual -> LN2 -> FFN(4x, relu) -> +residual

Full inputs in, full outputs out. Sharding: 8 cores = (batch b in 0..3) x (parity p in 0..1).
Core (b, p) owns query 512-blocks {2j+p : j in 0..1} of batch b (1024 tokens), computes K/V
for the whole batch (duplicated within the pair), runs block-causal attention with a uniform
SPMD program (per-core causal masks and exp-bias passed as data), then proj/LN2/FFN on its
own token rows. No collectives. Matmuls in float32r (TF32-like, 1 cyc/row at free-dim>=256).

Tricks:
 - LN gains/biases folded into weights host-side (wq' = g1*wq, qbias = wq^T be1, ...), so the
   transposed-layout LN apply is 2 DVE passes and Q/K/V get biases via the PSUM->SBUF copy.
 - Reciprocals (LN rstd, softmax denominator) are single-partition vectors; DVE reciprocal is
   ~6 cyc/elem/lane, so [1,N] is bounced through DRAM into [128,N/128], inverted lane-parallel,
   bounced back, and broadcast-loaded via a 0-stride DMA.
 - Fully-masked causal blocks are zeroed by a -30000 per-partition bias into exp (free);
   diagonal blocks get an elementwise 0/1 mask multiply (split between DVE and GpSimd).
 - x2 (post-attention residual) spills to DRAM; FFN re-reads it.
"""

import numpy as np

B, T, D = 4, 2048, 1024
H, DH = 16, 64
NG = 8            # head groups of 2 heads
TC = 1024         # tokens per core
QB = 512          # query block
NJ = 2            # local query blocks per core
F4 = 4096
EPS = 1e-5
SCALE = float(D) ** -0.5
N_CORES = 8

_cache = {}


def _build():
    import contextlib
    import concourse.bass as bass
    import concourse.mybir as mybir
    import concourse.tile as tile
    from concourse import bacc
    from concourse.masks import make_identity

    f32, f32r = mybir.dt.float32, mybir.dt.float32r
    bf16 = mybir.dt.bfloat16
    AF = mybir.ActivationFunctionType
    ALU = mybir.AluOpType

    nc = bacc.Bacc('TRN2', target_bir_lowering=False, debug=False,
                   num_devices=N_CORES)

    # ---- external I/O (per-core) ----
    xT_d = nc.dram_tensor("xT", [D, T], f32, kind="ExternalInput")
    xqT_d = nc.dram_tensor("xqT", [D, TC], f32, kind="ExternalInput")
    xq_d = nc.dram_tensor("xq", [TC, D], f32, kind="ExternalInput")
    wq_d = nc.dram_tensor("wqp", [NG, 8, 128, 128], f32, kind="ExternalInput")
    wk_d = nc.dram_tensor("wkp", [NG, 8, 128, 128], f32, kind="ExternalInput")
    wv_d = nc.dram_tensor("wvp", [NG, 8, 128, 128], f32, kind="ExternalInput")
    qb_d = nc.dram_tensor("qbias", [NG, 128], f32, kind="ExternalInput")
    kb_d = nc.dram_tensor("kbias", [NG, 128], f32, kind="ExternalInput")
    vb_d = nc.dram_tensor("vbias", [NG, 128], f32, kind="ExternalInput")
    wp_d = nc.dram_tensor("wp", [D, D], f32, kind="ExternalInput")
    w1_d = nc.dram_tensor("w1p", [32, 8, 128, 128], f32, kind="ExternalInput")
    w2_d = nc.dram_tensor("w2", [F4, D], f32, kind="ExternalInput")
    b1_d = nc.dram_tensor("b1t", [32, 128], f32, kind="ExternalInput")
    bp_d = nc.dram_tensor("bp", [D], f32, kind="ExternalInput")
    b2_d = nc.dram_tensor("b2", [D], f32, kind="ExternalInput")
    mk_d = nc.dram_tensor("masks", [8, 128, QB], f32, kind="ExternalInput")
    eb_d = nc.dram_tensor("expbias", [8], f32, kind="ExternalInput")
    out_d = nc.dram_tensor("out", [TC, D], f32, kind="ExternalOutput")

    x2_dram = nc.dram_tensor("x2_scratch", [TC, D], f32)
    den_dram = nc.dram_tensor("den_scratch", [2, TC], f32)
    rden_dram = nc.dram_tensor("rden_scratch", [2, TC], f32)
    mu_dram = nc.dram_tensor("mu_scratch", [2, 512], f32)
    sd_dram = nc.dram_tensor("sd_scratch", [2, 512], f32)
    rs_dram = nc.dram_tensor("rs_scratch", [2, 512], f32)

    def bcast_ap(dram_ap, parts, n):
        return bass.AP(tensor=dram_ap.tensor, offset=dram_ap.offset,
                       ap=[[0, parts], [1, n]])

    with tile.TileContext(nc) as tc:
        ctx = contextlib.ExitStack()
        with ctx:
            consts = ctx.enter_context(tc.tile_pool(name="consts", bufs=1))
            persist = ctx.enter_context(tc.tile_pool(name="persist", bufs=1))

            # ---------- constants ----------
            ident_f = consts.tile([128, 128], f32)
            make_identity(nc, ident_f)
            ident_r = consts.tile([128, 128], f32r)
            nc.vector.tensor_copy(ident_r, ident_f)
            ones_f = consts.tile([128, 16], f32)
            nc.vector.memset(ones_f, 1.0)
            ones_r = consts.tile([128, 1], f32r)
            nc.vector.tensor_copy(ones_r, ones_f[:, 0:1])
            eps_t = consts.tile([1, 1], f32)
            nc.vector.memset(eps_t, EPS)
            ebias_t = consts.tile([128, 8], f32)
            nc.sync.dma_start(out=ebias_t, in_=bcast_ap(eb_d[:], 128, 8))

            # ---------- transposed layernorm (in-place, 2-pass apply) ----------
            def ln_T(src_all, n_tok, wpool, spool, pspool):
                nch = n_tok // 512
                for c in range(nch):
                    sl = bass.ds(c * 512, 512)
                    cb = c % 2
                    mu_ps = pspool.tile([1, 512], f32, tag="mu_ps")
                    sq_ps = pspool.tile([1, 512], f32, tag="sq_ps")
                    for i in range(8):
                        sq = wpool.tile([128, 512], f32r, tag="sq")
                        nc.scalar.activation(out=sq, in_=src_all[:, i, sl].bitcast(f32),
                                             func=AF.Square)
                        nc.tensor.matmul(mu_ps, ones_r, src_all[:, i, sl],
                                         start=(i == 0), stop=(i == 7))
                        nc.tensor.matmul(sq_ps, ones_r, sq,
                                         start=(i == 0), stop=(i == 7))
                    mu = spool.tile([1, 512], f32, tag="mu")
                    nc.scalar.mul(mu, mu_ps, 1.0 / D)
                    sb = spool.tile([1, 512], f32, tag="sb")
                    nc.scalar.mul(sb, sq_ps, 1.0 / D)
                    sc = spool.tile([1, 512], f32, tag="sc")
                    nc.vector.tensor_tensor(out=sc, in0=mu, in1=mu, op=ALU.mult)
                    nc.vector.tensor_tensor(out=sb, in0=sb, in1=sc, op=ALU.subtract)
                    nc.scalar.activation(out=sb, in_=sb, func=AF.Sqrt, bias=eps_t)
                    # lane-parallel reciprocal via DRAM bounce
                    nc.sync.dma_start(out=mu_dram[cb, :], in_=mu)
                    nc.sync.dma_start(out=sd_dram[cb, :], in_=sb)
                    dd = spool.tile([128, 4], f32, tag="dd")
                    nc.sync.dma_start(
                        out=dd, in_=sd_dram[cb, :].rearrange("(p i) -> p i", p=128))
                    rr = spool.tile([128, 4], f32, tag="rr")
                    nc.vector.reciprocal(rr, dd)
                    nt = spool.tile([128, 4], f32, tag="nt")
                    nc.vector.tensor_tensor(out=nt, in0=dd, in1=rr, op=ALU.mult)
                    nc.vector.tensor_scalar(out=nt, in0=nt, scalar1=-1.0,
                                            scalar2=2.0, op0=ALU.mult, op1=ALU.add)
                    nc.vector.tensor_tensor(out=rr, in0=rr, in1=nt, op=ALU.mult)
                    nc.sync.dma_start(
                        out=rs_dram[cb, :].rearrange("(p i) -> p i", p=128), in_=rr)
                    mu_b = wpool.tile([128, 512], f32, tag="mu_b")
                    nc.sync.dma_start(out=mu_b, in_=bcast_ap(mu_dram[cb, :], 128, 512))
                    rstd_b = wpool.tile([128, 512], f32, tag="rstd_b")
                    nc.sync.dma_start(out=rstd_b,
                                      in_=bcast_ap(rs_dram[cb, :], 128, 512))
                    for i in range(8):
                        t1 = wpool.tile([128, 512], f32, tag="t1")
                        nc.vector.tensor_tensor(out=t1,
                                                in0=src_all[:, i, sl].bitcast(f32),
                                                in1=mu_b, op=ALU.subtract)
                        nc.vector.tensor_tensor(out=src_all[:, i, sl], in0=t1,
                                                in1=rstd_b, op=ALU.mult)

            # ---------- LN1: xqT first (small), then xT ----------
            # persist tags: t64: hT -> uT ; t32a: hqT -> x2T ; t32b: attT
            hqT = persist.tile([128, 8, TC], f32r, tag="t32a")
            hT = persist.tile([128, 8, T], f32r, tag="t64")
            with tc.tile_pool(name="ln_work", bufs=3) as lnw, \
                 tc.tile_pool(name="ln_stats", bufs=2) as lns, \
                 tc.tile_pool(name="ps_ln1", bufs=1, space="PSUM") as ps_ln1:
                for i in range(8):
                    nc.sync.dma_start(
                        out=hqT[:, i, :],
                        in_=xqT_d[i * 128:(i + 1) * 128, :].bitcast(f32r))
                ln_T(hqT, TC, lnw, lns, ps_ln1)
                for i in range(8):
                    nc.sync.dma_start(
                        out=hT[:, i, :],
                        in_=xT_d[i * 128:(i + 1) * 128, :].bitcast(f32r))
                ln_T(hT, T, lnw, lns, ps_ln1)

            # ---------- attention ----------
            attT = persist.tile([128, 8, TC], f32r, tag="t32b")
            with tc.tile_pool(name="kv_pool", bufs=1) as kvp, \
                 tc.tile_pool(name="wg_pool", bufs=1) as wgp, \
                 tc.tile_pool(name="att_work", bufs=3) as atw, \
                 tc.tile_pool(name="den_pool", bufs=1) as dnp, \
                 tc.tile_pool(name="ps_qkv", bufs=2, space="PSUM") as ps_qkv, \
                 tc.tile_pool(name="ps_st", bufs=3, space="PSUM") as ps_st, \
                 tc.tile_pool(name="ps_acc", bufs=1, space="PSUM") as ps_acc, \
                 tc.tile_pool(name="att_c", bufs=1) as attc:
                mask_t = attc.tile([128, 8, QB], f32)
                nc.sync.dma_start(out=mask_t,
                                  in_=mk_d[:, :, :].rearrange("r p q -> p r q"))
                qbias_t = attc.tile([128, 8], f32)
                nc.sync.dma_start(out=qbias_t, in_=qb_d[:, :].rearrange("g p -> p g"))
                kbias_t = attc.tile([128, 8], f32)
                nc.sync.dma_start(out=kbias_t, in_=kb_d[:, :].rearrange("g p -> p g"))
                vbias_t = attc.tile([128, 8], f32)
                nc.sync.dma_start(out=vbias_t, in_=vb_d[:, :].rearrange("g p -> p g"))
                for g in range(NG):
                    wqg = wgp.tile([128, 8, 128], f32r, tag="wqg")
                    nc.sync.dma_start(
                        out=wqg, in_=wq_d[g].rearrange("k p c -> p k c").bitcast(f32r))
                    wkg = wgp.tile([128, 8, 128], f32r, tag="wkg")
                    nc.sync.dma_start(
                        out=wkg, in_=wk_d[g].rearrange("k p c -> p k c").bitcast(f32r))
                    wvg = wgp.tile([128, 8, 128], f32r, tag="wvg")
                    nc.sync.dma_start(
                        out=wvg, in_=wv_d[g].rearrange("k p c -> p k c").bitcast(f32r))
                    kt = kvp.tile([128, T], bf16, tag="kt")
                    vt = kvp.tile([128, T], f32r, tag="vt")
                    qt = kvp.tile([128, TC], bf16, tag="qt")
                    for n in range(4):
                        sl = bass.ds(n * 512, 512)
                        psk = ps_qkv.tile([128, 512], f32, tag="qkv")
                        for k in range(8):
                            nc.tensor.matmul(psk, wkg[:, k, :], hT[:, k, sl],
                                             start=(k == 0), stop=(k == 7))
                        nc.vector.tensor_scalar_add(kt[:, sl], psk,
                                                    kbias_t[:, g:g + 1])
                        psv = ps_qkv.tile([128, 512], f32, tag="qkv")
                        for k in range(8):
                            nc.tensor.matmul(psv, wvg[:, k, :], hT[:, k, sl],
                                             start=(k == 0), stop=(k == 7))
                        nc.vector.tensor_scalar_add(vt[:, sl], psv,
                                                    vbias_t[:, g:g + 1])
                    for n in range(2):
                        sl = bass.ds(n * 512, 512)
                        psq = ps_qkv.tile([128, 512], f32, tag="qkv")
                        for k in range(8):
                            nc.tensor.matmul(psq, wqg[:, k, :], hqT[:, k, sl],
                                             start=(k == 0), stop=(k == 7))
                        nc.vector.tensor_scalar_add(qt[:, sl], psq,
                                                    qbias_t[:, g:g + 1])
                    # V natural per head, with ones column (col 64)
                    vxa = kvp.tile([128, 16, 65], f32r, tag="vxa")
                    vxb = kvp.tile([128, 16, 65], f32r, tag="vxb")
                    nc.vector.tensor_copy(vxa[:, :, 64:65], ones_f.bitcast(f32r))
                    nc.vector.tensor_copy(vxb[:, :, 64:65], ones_f.bitcast(f32r))
                    for kt_i in range(16):
                        pst = ps_tr.tile([128, 128], f32r, tag="tr")
                        nc.tensor.transpose(pst, vt[:, kt_i * 128:(kt_i + 1) * 128],
                                            ident_r)
                        nc.vector.tensor_copy(vxa[:, kt_i, 0:64], pst[:, 0:64])
                        nc.vector.tensor_copy(vxb[:, kt_i, 0:64], pst[:, 64:128])
                    dens = []
                    for h in range(2):
                        den_t = dnp.tile([1, TC], f32, tag=f"den{h}")
                        dens.append(den_t)
                    for j in range(NJ):
                        nkb = 8 * j + 8
                        qsl = bass.ds(j * QB, QB)
                        accs = []
                        for h in range(2):
                            acc_t = ps_acc.tile([65, QB], f32, tag=f"acc{h}")
                            accs.append(acc_t)
                        for kb in range(nkb):
                            rel = kb - 8 * j
                            for h in range(2):
                                hs = bass.ds(h * 64, 64)
                                st = ps_st.tile([128, QB], f32, tag="st")
                                nc.tensor.matmul(
                                    st, kt[hs, kb * 128:(kb + 1) * 128],
                                    qt[hs, qsl], start=True, stop=True,
                                    tile_position=(h * 64, 0))
                                pt = atw.tile([128, QB], f32r, tag="pt")
                                if rel < 0:
                                    nc.scalar.activation(out=pt, in_=st,
                                                         func=AF.Exp, scale=SCALE)
                                else:
                                    nc.scalar.activation(
                                        out=pt, in_=st, func=AF.Exp, scale=SCALE,
                                        bias=ebias_t[:, rel:rel + 1])
                                    if rel < 4:
                                        nc.vector.tensor_tensor(
                                            out=pt, in0=pt.bitcast(f32),
                                            in1=mask_t[:, rel, :], op=ALU.mult)
                                    else:
                                        nc.gpsimd.tensor_mul(
                                            out=pt, in0=pt.bitcast(f32),
                                            in1=mask_t[:, rel, :])
                                vx = vxa if h == 0 else vxb
                                nc.tensor.matmul(accs[h], vx[:, kb, :], pt,
                                                 start=(kb == 0),
                                                 stop=(kb == nkb - 1))
                        for h in range(2):
                            hs = bass.ds(h * 64, 64)
                            nc.vector.tensor_copy(attT[hs, g, qsl],
                                                  accs[h][0:64, :])
                            nc.scalar.copy(dens[h][0:1, qsl], accs[h][64:65, :])
                    for h in range(2):
                        hs = bass.ds(h * 64, 64)
                        nc.sync.dma_start(out=den_dram[h, :], in_=dens[h][0:1, :])
                        dd = dnp.tile([128, TC // 128], f32, tag="dd")
                        nc.sync.dma_start(
                            out=dd,
                            in_=den_dram[h, :].rearrange("(p i) -> p i", p=128))
                        rr = dnp.tile([128, TC // 128], f32, tag="rr")
                        nc.vector.reciprocal(rr, dd)
                        nt2 = dnp.tile([128, TC // 128], f32, tag="nt2")
                        nc.vector.tensor_tensor(out=nt2, in0=dd, in1=rr,
                                                op=ALU.mult)
                        nc.vector.tensor_scalar(out=nt2, in0=nt2, scalar1=-1.0,
                                                scalar2=2.0, op0=ALU.mult,
                                                op1=ALU.add)
                        nc.vector.tensor_tensor(out=rr, in0=rr, in1=nt2,
                                                op=ALU.mult)
                        nc.sync.dma_start(
                            out=rden_dram[h, :].rearrange("(p i) -> p i", p=128),
                            in_=rr)
                        rb = dnp.tile([128, TC], f32, tag="rb")
                        nc.sync.dma_start(out=rb,
                                          in_=bcast_ap(rden_dram[h, :], 128, TC))
                        nc.vector.tensor_tensor(out=attT[hs, g, :],
                                                in0=attT[hs, g, :].bitcast(f32),
                                                in1=rb[hs, :], op=ALU.mult)

            # ---------- proj + residual -> x2 (spilled to DRAM) ----------
            with tc.tile_pool(name="proj_pool", bufs=2) as prp, \
                 tc.tile_pool(name="proj_c", bufs=1) as prc, \
                 tc.tile_pool(name="ps_proj", bufs=2, space="PSUM") as ps_proj:
                bp_b = prc.tile([128, D], f32)
                nc.sync.dma_start(out=bp_b, in_=bcast_ap(bp_d[:], 128, D))
                wp_sb = prc.tile([128, 8, D], f32r)
                nc.sync.dma_start(
                    out=wp_sb,
                    in_=wp_d[:, :].rearrange("(k p) o -> p k o", p=128).bitcast(f32r))
                for mt in range(8):
                    xqt = prp.tile([128, D], f32, tag="xq")
                    nc.sync.dma_start(out=xqt, in_=xq_d[mt * 128:(mt + 1) * 128, :])
                    for oc in range(2):
                        osl = bass.ds(oc * 512, 512)
                        ps = ps_proj.tile([128, 512], f32, tag="proj")
                        for k in range(8):
                            nc.tensor.matmul(ps, attT[:, k, mt * 128:(mt + 1) * 128],
                                             wp_sb[:, k, osl],
                                             start=(k == 0), stop=(k == 7))
                        tt = prp.tile([128, 512], f32, tag="tt")
                        nc.vector.tensor_tensor(out=tt, in0=ps, in1=xqt[:, osl],
                                                op=ALU.add)
                        nc.vector.tensor_tensor(out=tt, in0=tt, in1=bp_b[:, osl],
                                                op=ALU.add)
                        nc.sync.dma_start(
                            out=x2_dram[mt * 128:(mt + 1) * 128,
                                        oc * 512:(oc + 1) * 512],
                            in_=tt)

            # ---------- LN2 (transposed) + FFN per 512-token chunk ----------
            with tc.tile_pool(name="ffn_c", bufs=1) as fcc, \
                 tc.tile_pool(name="ffn_x", bufs=2) as fx, \
                 tc.tile_pool(name="ffn_w", bufs=3) as fw, \
                 tc.tile_pool(name="ln2_work", bufs=2) as lnw2, \
                 tc.tile_pool(name="ln2_stats", bufs=1) as lns2, \
                 tc.tile_pool(name="ps_ln2", bufs=1, space="PSUM") as ps_ln2, \
                 tc.tile_pool(name="ps_u", bufs=2, space="PSUM") as ps_u, \
                 tc.tile_pool(name="ps_v", bufs=1, space="PSUM") as ps_v:
                b1t = fcc.tile([128, 32], f32)
                nc.sync.dma_start(out=b1t, in_=b1_d[:, :].rearrange("i p -> p i"))
                b2_b = fcc.tile([128, D], f32)
                nc.sync.dma_start(out=b2_b, in_=bcast_ap(b2_d[:], 128, D))
                for tcx in range(2):
                    x2T = persist.tile([128, 8, 512], f32r, tag="t32a")
                    for mtl in range(4):
                        x2ld = fx.tile([128, D], f32, tag="x2ld")
                        mt = tcx * 4 + mtl
                        nc.sync.dma_start(out=x2ld,
                                          in_=x2_dram[mt * 128:(mt + 1) * 128, :])
                        for i in range(8):
                            pst = ps_u.tile([128, 128], f32, tag="u")
                            nc.tensor.transpose(pst, x2ld[:, i * 128:(i + 1) * 128],
                                                ident_f)
                            nc.vector.tensor_copy(
                                x2T[:, i, mtl * 128:(mtl + 1) * 128], pst)
                    ln_T(x2T, 512, lnw2, lns2, ps_ln2)
                    uT = persist.tile([128, 32, 512], f32r, tag="t64")
                    for i in range(32):
                        w1t = fw.tile([128, 8, 128], f32r, tag="w1t")
                        nc.sync.dma_start(
                            out=w1t,
                            in_=w1_d[i].rearrange("k p c -> p k c").bitcast(f32r))
                        psu = ps_u.tile([128, 512], f32, tag="u")
                        for k in range(8):
                            nc.tensor.matmul(psu, w1t[:, k, :], x2T[:, k, :],
                                             start=(k == 0), stop=(k == 7))
                        nc.scalar.activation(out=uT[:, i, :], in_=psu, func=AF.Relu,
                                             bias=b1t[:, i:i + 1])
                    for oc in range(2):
                        osl = bass.ds(oc * 512, 512)
                        psv = []
                        for mtl in range(4):
                            psv_t = ps_v.tile([128, 512], f32, tag=f"v{mtl}")
                            psv.append(psv_t)
                        for i in range(32):
                            w2t = fw.tile([128, 512], f32r, tag="w2t")
                            nc.sync.dma_start(
                                out=w2t,
                                in_=w2_d[i * 128:(i + 1) * 128,
                                         oc * 512:(oc + 1) * 512].bitcast(f32r))
                            for mtl in range(4):
                                nc.tensor.matmul(
                                    psv[mtl], uT[:, i, mtl * 128:(mtl + 1) * 128],
                                    w2t, start=(i == 0), stop=(i == 31))
                        for mtl in range(4):
                            mt = tcx * 4 + mtl
                            x2r = fx.tile([128, 512], f32, tag="x2r")
                            nc.sync.dma_start(
                                out=x2r,
                                in_=x2_dram[mt * 128:(mt + 1) * 128,
                                            oc * 512:(oc + 1) * 512])
                            ot = fx.tile([128, 512], f32, tag="ot")
                            nc.vector.tensor_tensor(out=ot, in0=psv[mtl],
                                                    in1=b2_b[:, osl], op=ALU.add)
                            nc.vector.tensor_tensor(out=ot, in0=ot, in1=x2r,
                                                    op=ALU.add)
                            nc.sync.dma_start(
                                out=out_d[mt * 128:(mt + 1) * 128,
                                          oc * 512:(oc + 1) * 512],
                                in_=ot)

    nc.compile()
    return nc


def _prep_shared(wq, wk, wv, wp, bp, w1, b1, w2, b2, g1, be1, g2, be2):
    c = np.ascontiguousarray
    f = np.float32

    def cf(a):
        return c(np.asarray(a, f))

    g1 = cf(g1)
    be1 = cf(be1)
    g2 = cf(g2)
    be2 = cf(be2)
    wq0, wk0, wv0, w10 = cf(wq), cf(wk), cf(wv), cf(w1)
    # fold LN gains into weights; LN biases become post-projection biases
    wqs = wq0 * g1[:, None]
    wks = wk0 * g1[:, None]
    wvs = wv0 * g1[:, None]
    w1s = w10 * g2[:, None]
    qbias = (wq0.T @ be1).astype(f)
    kbias = (wk0.T @ be1).astype(f)
    vbias = (wv0.T @ be1).astype(f)
    b1n = (cf(b1) + w10.T @ be2).astype(f)
    return {
        "wqp": c(wqs.reshape(8, 128, 8, 128).transpose(2, 0, 1, 3)),
        "wkp": c(wks.reshape(8, 128, 8, 128).transpose(2, 0, 1, 3)),
        "wvp": c(wvs.reshape(8, 128, 8, 128).transpose(2, 0, 1, 3)),
        "qbias": c(qbias.reshape(NG, 128)),
        "kbias": c(kbias.reshape(NG, 128)),
        "vbias": c(vbias.reshape(NG, 128)),
        "wp": cf(wp),
        "w1p": c(w1s.reshape(8, 128, 32, 128).transpose(2, 0, 1, 3)),
        "w2": cf(w2),
        "b1t": c(b1n.reshape(32, 128)),
        "bp": cf(bp),
        "b2": cf(b2),
    }


def _own_idx(p):
    return (np.arange(NJ)[:, None] * 1024 + p * QB + np.arange(QB)[None, :]).ravel()


def _masks(p):
    m = np.zeros((8, 128, QB), np.float32)
    k = np.arange(128)[:, None]
    q = np.arange(QB)[None, :]
    for rel in range(8):
        m[rel] = (128 * rel + k <= QB * p + q).astype(np.float32)
    return m


def _expbias(p):
    e = np.zeros(8, np.float32)
    if p == 0:
        e[4:] = -30000.0
    return e


def _make_in_maps(x, shared):
    in_maps = []
    for c in range(N_CORES):
        b, p = c // 2, c % 2
        xb = np.asarray(x[b], np.float32)
        idx = _own_idx(p)
        xq = np.ascontiguousarray(xb[idx])
        m = dict(shared)
        m["xT"] = np.ascontiguousarray(xb.T)
        m["xq"] = xq
        m["xqT"] = np.ascontiguousarray(xq.T)
        m["masks"] = _masks(p)
        m["expbias"] = _expbias(p)
        in_maps.append(m)
    return in_maps


def kernel(**inputs):
    from concourse.bass_utils import run_bass_kernel_spmd

    if "nc" not in _cache:
        _cache["nc"] = _build()
    nc = _cache["nc"]

    shared = _prep_shared(
        inputs["wq"], inputs["wk"], inputs["wv"], inputs["wp"], inputs["bp"],
        inputs["w1"], inputs["b1"], inputs["w2"], inputs["b2"],
        inputs["g1"], inputs["be1"], inputs["g2"], inputs["be2"])
    in_maps = _make_in_maps(inputs["x"], shared)

    res = run_bass_kernel_spmd(nc, in_maps, list(range(N_CORES)))
    out = np.empty((B, T, D), np.float32)
    for c in range(N_CORES):
        b, p = c // 2, c % 2
        out[b][_own_idx(p)] = res.results[c]["out"]
    return out
